# revision 13
# baseline (speedup 1.0000x reference)
"""Trainium2 Bass kernel for BarycentricCoordinates — v5.

v2's per-c Delaunay restructure + manual software pipelining: the two
r-chunks of a v-tile and the NEXT v-tile's per-c precompute are issued
round-robin, one instruction per slot, so consecutive ops on each engine
come from independent chains (hides DVE pipeline latency that serialized
v2 on HW). Arithmetic is identical to v2 (bitwise-matches the reference).
"""

import sys

sys.path.insert(0, "/opt/trn_rl_repo")

import numpy as np

import concourse.bass as bass
import concourse.bacc as bacc
import concourse.mybir as mybir
from concourse.tile import TileContext

F32 = mybir.dt.float32
I32 = mybir.dt.int32
OP = mybir.AluOpType
AF = mybir.ActivationFunctionType
AX = mybir.AxisListType

BIG = 2.0e38
N_CORES = 8
V_TOTAL = 5000
R, A, K0 = 5, 8, 8
RA = R * A
VS = V_TOTAL // N_CORES
P = 128
VSP = 640
RC = 20
K2 = 64
K3 = 512


def build_nc(vsp=VSP, rc=RC, ra=RA):
    nc = bacc.Bacc("TRN2", target_bir_lowering=False)
    n_vt = vsp // P
    n_rch = ra // rc

    px_d = nc.dram_tensor("px", (vsp, K0), F32, kind="ExternalInput")
    py_d = nc.dram_tensor("py", (vsp, K0), F32, kind="ExternalInput")
    tmpl_d = nc.dram_tensor("tmpl", (2, ra), F32, kind="ExternalInput")
    iota8_d = nc.dram_tensor("iota8", (1, K0), F32, kind="ExternalInput")
    iota64_d = nc.dram_tensor("iota64", (1, K2), F32, kind="ExternalInput")
    pow2_d = nc.dram_tensor("pow2", (1, K0), F32, kind="ExternalInput")
    outw_d = nc.dram_tensor("outw", (vsp, ra, 3), F32, kind="ExternalOutput")
    outi_d = nc.dram_tensor("outi", (vsp, ra, 3), F32, kind="ExternalOutput")

    with TileContext(nc) as tc:
        VE = nc.vector
        GP = nc.gpsimd
        SC = nc.scalar
        PP = rc * K2
        RK = rc * K0

        with (
            tc.tile_pool(name="const", bufs=1) as cpool,
            tc.tile_pool(name="vt", bufs=2) as vpool,
            tc.tile_pool(name="det", bufs=1) as spool,
            tc.tile_pool(name="pair", bufs=2) as ppool,
            tc.tile_pool(name="rk", bufs=2) as rkpool,
            tc.tile_pool(name="small", bufs=2) as opool,
        ):
            TX = cpool.tile([P, ra], F32, tag="TX")
            TY = cpool.tile([P, ra], F32, tag="TY")
            IOTA8 = cpool.tile([P, K0], F32, tag="IOTA8")
            IOTA64 = cpool.tile([P, K2], F32, tag="IOTA64")
            POW2 = cpool.tile([P, K0], F32, tag="POW2")
            nc.sync.dma_start(TX, tmpl_d[0:1, :].to_broadcast((P, ra)))
            nc.sync.dma_start(TY, tmpl_d[1:2, :].to_broadcast((P, ra)))
            nc.sync.dma_start(IOTA8, iota8_d[0:1, :].to_broadcast((P, K0)))
            nc.sync.dma_start(IOTA64, iota64_d[0:1, :].to_broadcast((P, K2)))
            nc.sync.dma_start(POW2, pow2_d[0:1, :].to_broadcast((P, K0)))

            def bcv(ap, shape):
                return ap.to_broadcast(shape)

            p0s = {}

            def p0_gen(vt):
                """Per-v-tile precompute: s, b-tensors, U, det(c,ij,k), bit-pack."""
                v0_, v1_ = vt * P, (vt + 1) * P
                px = vpool.tile([P, K0], F32, tag="px")
                py = vpool.tile([P, K0], F32, tag="py")
                nc.sync.dma_start(px, px_d[v0_:v1_, :])
                nc.sync.dma_start(py, py_d[v0_:v1_, :])
                yield
                s_ = vpool.tile([P, K0], F32, tag="s")
                t8 = vpool.tile([P, K0], F32, tag="t8")
                SC.activation(out=s_, in_=px, func=AF.Square)
                SC.activation(out=t8, in_=py, func=AF.Square)
                yield
                VE.tensor_tensor(out=s_, in0=s_, in1=t8, op=OP.add)
                yield
                PXYS = vpool.tile([P, 24], F32, tag="PXYS")
                SC.copy(out=PXYS[:, 0:8], in_=px)
                SC.copy(out=PXYS[:, 8:16], in_=py)
                SC.copy(out=PXYS[:, 16:24], in_=IOTA8)
                yield
                bx = vpool.tile([P, K2], F32, tag="bx")
                by = vpool.tile([P, K2], F32, tag="by")
                bs = vpool.tile([P, K2], F32, tag="bs")
                bxv = bx.rearrange("p (i k) -> p i k", k=K0)
                byv = by.rearrange("p (i k) -> p i k", k=K0)
                bsv = bs.rearrange("p (i k) -> p i k", k=K0)
                VE.tensor_tensor(out=bxv, in0=bcv(px.unsqueeze(2), (P, K0, K0)),
                                 in1=bcv(px.unsqueeze(1), (P, K0, K0)),
                                 op=OP.subtract)
                yield
                GP.tensor_tensor(out=byv, in0=bcv(py.unsqueeze(2), (P, K0, K0)),
                                 in1=bcv(py.unsqueeze(1), (P, K0, K0)),
                                 op=OP.subtract)
                yield
                VE.tensor_tensor(out=bsv, in0=bcv(s_.unsqueeze(2), (P, K0, K0)),
                                 in1=bcv(s_.unsqueeze(1), (P, K0, K0)),
                                 op=OP.subtract)
                yield

                def Bi(t):
                    return bcv(t.rearrange("p (i k) -> p i k", k=K0).unsqueeze(2),
                               (P, K0, K0, K0))

                def Bj(t):
                    return bcv(t.rearrange("p (j k) -> p j k", k=K0).unsqueeze(1),
                               (P, K0, K0, K0))

                U1 = vpool.tile([P, K3], F32, tag="U1")
                U2 = vpool.tile([P, K3], F32, tag="U2")
                U3 = vpool.tile([P, K3], F32, tag="U3")
                uA = vpool.tile([P, K3], F32, tag="uA")
                U1v = U1.rearrange("p (i j k) -> p i j k", j=K0, k=K0)
                U2v = U2.rearrange("p (i j k) -> p i j k", j=K0, k=K0)
                U3v = U3.rearrange("p (i j k) -> p i j k", j=K0, k=K0)
                uAv = uA.rearrange("p (i j k) -> p i j k", j=K0, k=K0)
                VE.tensor_tensor(out=U1v, in0=Bi(by), in1=Bj(bs), op=OP.mult)
                yield
                GP.tensor_tensor(out=uAv, in0=Bi(bs), in1=Bj(by), op=OP.mult)
                yield
                VE.tensor_tensor(out=U1, in0=U1, in1=uA, op=OP.subtract)
                yield
                VE.tensor_tensor(out=U2v, in0=Bi(bx), in1=Bj(bs), op=OP.mult)
                yield
                GP.tensor_tensor(out=uAv, in0=Bi(bs), in1=Bj(bx), op=OP.mult)
                yield
                VE.tensor_tensor(out=U2, in0=U2, in1=uA, op=OP.subtract)
                yield
                VE.tensor_tensor(out=U3v, in0=Bi(bx), in1=Bj(by), op=OP.mult)
                yield
                GP.tensor_tensor(out=uAv, in0=Bi(by), in1=Bj(bx), op=OP.mult)
                yield
                VE.tensor_tensor(out=U3, in0=U3, in1=uA, op=OP.subtract)
                yield

                det = spool.tile([P, K0 * K3], F32, tag="det")
                dtm = spool.tile([P, K0 * K3], F32, tag="dtm")
                detv = det.rearrange("p (c q k) -> p c q k", q=K2, k=K0)
                dtmv = dtm.rearrange("p (c q k) -> p c q k", q=K2, k=K0)

                def Ck(t):
                    return bcv(t.rearrange("p (c k) -> p c k", k=K0).unsqueeze(2),
                               (P, K0, K2, K0))

                def Uq(t):
                    return bcv(t.rearrange("p (q k) -> p q k", k=K0).unsqueeze(1),
                               (P, K0, K2, K0))

                VE.tensor_tensor(out=detv, in0=Ck(bx), in1=Uq(U1), op=OP.mult)
                yield
                GP.tensor_tensor(out=dtmv, in0=Ck(by), in1=Uq(U2), op=OP.mult)
                yield
                VE.tensor_tensor(out=det, in0=det, in1=dtm, op=OP.subtract)
                yield
                VE.tensor_tensor(out=dtmv, in0=Ck(bs), in1=Uq(U3), op=OP.mult)
                yield
                VE.tensor_tensor(out=det, in0=det, in1=dtm, op=OP.add)
                yield
                VD = vpool.tile([P, K3], F32, tag="VD")
                VDv = VD.rearrange("p (c q) -> p c q", q=K2)
                VE.tensor_reduce(out=VDv, in_=detv, axis=AX.X, op=OP.max)
                yield
                mpb = vpool.tile([P, K3], F32, tag="uA")
                mpbv = mpb.rearrange("p (c q) -> p c q", q=K2)
                VE.scalar_tensor_tensor(out=mpbv, in0=VDv, scalar=0.0,
                                        in1=bcv(POW2.unsqueeze(2), (P, K0, K2)),
                                        op0=OP.is_gt, op1=OP.mult)
                yield
                Mp = vpool.tile([P, K2], F32, tag="Mp")
                VE.tensor_reduce(out=Mp,
                                 in_=mpb.rearrange("p (c q) -> p q c", q=K2),
                                 axis=AX.X, op=OP.add)
                yield
                ipack = vpool.tile([P, K2], I32, tag="ipack")
                VE.tensor_copy(out=ipack, in_=Mp)
                yield
                p0s[vt] = dict(px=px, py=py, PXYS=PXYS, ipack=ipack)

            def chunk_gen(vt, rchunk):
                """Per-(v-tile, r-chunk): closest, weights, score, selection."""
                v0_, v1_ = vt * P, (vt + 1) * P
                S = p0s[vt]
                px, py, PXYS, ipack = S["px"], S["py"], S["PXYS"], S["ipack"]
                r0 = rchunk * rc
                r1 = r0 + rc

                d2 = rkpool.tile([P, RK], F32, tag="d2")
                tdx = rkpool.tile([P, RK], F32, tag="tdx")
                tdy = rkpool.tile([P, RK], F32, tag="tdy")
                d2v = d2.rearrange("p (r k) -> p r k", k=K0)
                tdxv = tdx.rearrange("p (r k) -> p r k", k=K0)
                tdyv = tdy.rearrange("p (r k) -> p r k", k=K0)
                px_rk = bcv(px.unsqueeze(1), (P, rc, K0))
                py_rk = bcv(py.unsqueeze(1), (P, rc, K0))
                tx_rk = bcv(TX[:, r0:r1].unsqueeze(2), (P, rc, K0))
                ty_rk = bcv(TY[:, r0:r1].unsqueeze(2), (P, rc, K0))
                VE.tensor_tensor(out=tdxv, in0=px_rk, in1=tx_rk, op=OP.subtract)
                yield
                GP.tensor_tensor(out=tdyv, in0=py_rk, in1=ty_rk, op=OP.subtract)
                yield
                SC.activation(out=tdx, in_=tdx, func=AF.Square)
                yield
                SC.activation(out=tdy, in_=tdy, func=AF.Square)
                yield
                VE.tensor_tensor(out=d2, in0=tdx, in1=tdy, op=OP.add)
                yield
                dmin = opool.tile([P, rc], F32, tag="dmin")
                VE.tensor_reduce(out=dmin, in_=d2v, axis=AX.X, op=OP.min)
                yield
                dmin_rk = bcv(dmin.unsqueeze(2), (P, rc, K0))
                m0 = rkpool.tile([P, RK], F32, tag="m0")
                m0v = m0.rearrange("p (r k) -> p r k", k=K0)
                VE.tensor_tensor(out=m0v, in0=d2v, in1=dmin_rk, op=OP.is_equal)
                yield
                tA3 = rkpool.tile([P, RK * 3], F32, tag="tA3")
                tA3v = tA3.rearrange("p (r g k) -> p r g k", g=3, k=K0)
                tA3r = rkpool.tile([P, rc * 3], F32, tag="tA3r")
                tA3rv = tA3r.rearrange("p (r g) -> p r g", g=3)
                m0_rgk = bcv(m0v.unsqueeze(2), (P, rc, 3, K0))
                pxys_rgk = bcv(PXYS.rearrange("p (g k) -> p g k", k=K0)
                               .unsqueeze(1), (P, rc, 3, K0))
                GP.tensor_tensor(out=tA3v, in0=m0_rgk, in1=pxys_rgk, op=OP.mult)
                yield
                VE.tensor_reduce(out=tA3rv, in_=tA3v, axis=AX.X, op=OP.add)
                yield
                cx = tA3rv[:, :, 0:1].squeeze(2)
                cy = tA3rv[:, :, 1:2].squeeze(2)
                c_f = tA3rv[:, :, 2:3].squeeze(2)

                d2b = rkpool.tile([P, RK], F32, tag="d2b")
                VE.scalar_tensor_tensor(out=d2b, in0=m0, scalar=BIG, in1=d2,
                                        op0=OP.mult, op1=OP.add)
                yield
                dmin2 = opool.tile([P, rc], F32, tag="dmin2")
                d2bv = d2b.rearrange("p (r k) -> p r k", k=K0)
                VE.tensor_reduce(out=dmin2, in_=d2bv, axis=AX.X, op=OP.min)
                yield
                dmin2_rk = bcv(dmin2.unsqueeze(2), (P, rc, K0))
                tA = rkpool.tile([P, RK], F32, tag="tA")
                tAv = tA.rearrange("p (r k) -> p r k", k=K0)
                i8_rk = bcv(IOTA8.unsqueeze(1), (P, rc, K0))
                VE.tensor_tensor(out=tAv, in0=d2bv, in1=dmin2_rk, op=OP.is_equal)
                yield
                GP.tensor_tensor(out=tAv, in0=tAv, in1=i8_rk, op=OP.mult)
                yield
                o1_f = opool.tile([P, rc], F32, tag="o1_f")
                VE.tensor_reduce(out=o1_f, in_=tAv, axis=AX.X, op=OP.add)
                yield

                v0x = rkpool.tile([P, RK], F32, tag="v0x")
                v0y = rkpool.tile([P, RK], F32, tag="v0y")
                d00 = rkpool.tile([P, RK], F32, tag="d00")
                d02 = rkpool.tile([P, RK], F32, tag="d02")
                tB = rkpool.tile([P, RK], F32, tag="tB")
                v0xv = v0x.rearrange("p (r k) -> p r k", k=K0)
                v0yv = v0y.rearrange("p (r k) -> p r k", k=K0)
                d02v = d02.rearrange("p (r k) -> p r k", k=K0)
                tBv = tB.rearrange("p (r k) -> p r k", k=K0)
                cx_rk = bcv(cx.unsqueeze(2), (P, rc, K0))
                cy_rk = bcv(cy.unsqueeze(2), (P, rc, K0))
                VE.tensor_tensor(out=v0xv, in0=px_rk, in1=cx_rk, op=OP.subtract)
                yield
                GP.tensor_tensor(out=v0yv, in0=py_rk, in1=cy_rk, op=OP.subtract)
                yield
                v2x = opool.tile([P, rc], F32, tag="v2x")
                v2y = opool.tile([P, rc], F32, tag="v2y")
                VE.tensor_tensor(out=v2x, in0=TX[:, r0:r1], in1=cx, op=OP.subtract)
                VE.tensor_tensor(out=v2y, in0=TY[:, r0:r1], in1=cy, op=OP.subtract)
                yield
                SC.activation(out=d00, in_=v0x, func=AF.Square)
                yield
                SC.activation(out=tB, in_=v0y, func=AF.Square)
                yield
                VE.tensor_tensor(out=d00, in0=d00, in1=tB, op=OP.add)
                yield
                v2x_rk = bcv(v2x.unsqueeze(2), (P, rc, K0))
                v2y_rk = bcv(v2y.unsqueeze(2), (P, rc, K0))
                VE.tensor_tensor(out=d02v, in0=v0xv, in1=v2x_rk, op=OP.mult)
                yield
                GP.tensor_tensor(out=tBv, in0=v0yv, in1=v2y_rk, op=OP.mult)
                yield
                VE.tensor_tensor(out=d02, in0=d02, in1=tB, op=OP.add)
                yield

                def XI(t2):
                    return bcv(t2.rearrange("p (r k) -> p r k", k=K0).unsqueeze(3),
                               (P, rc, K0, K0))

                def XJ(t2):
                    return bcv(t2.rearrange("p (r k) -> p r k", k=K0).unsqueeze(2),
                               (P, rc, K0, K0))

                dot01 = ppool.tile([P, PP], F32, tag="dot01")
                pA = ppool.tile([P, PP], F32, tag="pA")
                pB = ppool.tile([P, PP], F32, tag="pB")
                w2t = ppool.tile([P, PP], F32, tag="w2t")
                w0t = ppool.tile([P, PP], F32, tag="w0t")
                inv = ppool.tile([P, PP], F32, tag="inv")
                dot01v = dot01.rearrange("p (r i j) -> p r i j", i=K0, j=K0)
                pAv = pA.rearrange("p (r i j) -> p r i j", i=K0, j=K0)
                pBv = pB.rearrange("p (r i j) -> p r i j", i=K0, j=K0)

                VE.tensor_tensor(out=dot01v, in0=XI(v0x), in1=XJ(v0x), op=OP.mult)
                yield
                GP.tensor_tensor(out=pAv, in0=XI(v0y), in1=XJ(v0y), op=OP.mult)
                yield
                VE.tensor_tensor(out=dot01, in0=dot01, in1=pA, op=OP.add)
                yield
                GP.tensor_tensor(out=pAv, in0=XI(d00), in1=XJ(d00), op=OP.mult)
                yield
                SC.activation(out=pB, in_=dot01, func=AF.Square)
                yield
                VE.tensor_tensor(out=pA, in0=pA, in1=pB, op=OP.subtract)  # denom
                yield
                VE.reciprocal(out=inv, in_=pA)
                yield
                VE.tensor_scalar(out=inv, in0=inv, scalar1=BIG, scalar2=-BIG,
                                 op0=OP.min, op1=OP.max)
                yield
                VE.tensor_tensor(out=pAv, in0=XJ(d00), in1=XI(d02), op=OP.mult)
                yield
                GP.tensor_tensor(out=pBv, in0=dot01v, in1=XJ(d02), op=OP.mult)
                yield
                VE.tensor_tensor(out=w2t, in0=pA, in1=pB, op=OP.subtract)
                yield
                VE.tensor_tensor(out=w2t, in0=w2t, in1=inv, op=OP.mult)
                yield
                w2tv = w2t.rearrange("p (r i j) -> p r i j", i=K0, j=K0)
                w1t = w2t.rearrange("p (r i j) -> p r j i", i=K0, j=K0)
                VE.tensor_tensor(out=pAv, in0=w2tv, in1=w1t, op=OP.add)
                yield
                SC.activation(out=w0t, in_=pA, func=AF.Copy, bias=1.0, scale=-1.0)
                yield
                wm = ppool.tile([P, PP], F32, tag="wm")
                wmv = wm.rearrange("p (r i j) -> p r i j", i=K0, j=K0)
                VE.tensor_tensor(out=wmv, in0=w1t, in1=w2tv, op=OP.min)
                yield
                VE.tensor_tensor(out=wm, in0=wm, in1=w0t, op=OP.min)
                yield
                sq = ppool.tile([P, PP], F32, tag="sq")
                sr = ppool.tile([P, PP], F32, tag="sr")
                SC.activation(out=sr, in_=w0t, func=AF.Square)
                yield
                SC.activation(out=sq, in_=w2t, func=AF.Square)
                yield
                VE.tensor_tensor(out=sr, in0=sr, in1=sq, op=OP.max)
                yield
                srv = sr.rearrange("p (r i j) -> p r i j", i=K0, j=K0)
                VE.tensor_tensor(out=srv, in0=srv,
                                 in1=sq.rearrange("p (r i j) -> p r j i",
                                                  i=K0, j=K0), op=OP.max)
                yield
                VE.tensor_scalar(out=pB, in0=wm, scalar1=0.0, scalar2=BIG,
                                 op0=OP.is_le, op1=OP.mult)
                yield
                score = ppool.tile([P, PP], F32, tag="score")
                VE.tensor_tensor(out=score, in0=sr, in1=pB, op=OP.max)
                yield
                c_i = opool.tile([P, rc], I32, tag="c_i")
                VE.tensor_copy(out=c_i, in_=c_f)
                yield
                shf = ppool.tile([P, PP], F32, tag="pB")
                sh = shf.bitcast(I32)
                shv = sh.rearrange("p (r q) -> p r q", q=K2)
                VE.tensor_tensor(out=shv,
                                 in0=bcv(ipack.unsqueeze(1), (P, rc, K2)),
                                 in1=bcv(c_i.unsqueeze(2), (P, rc, K2)),
                                 op=OP.arith_shift_right)
                yield
                VE.tensor_scalar(out=sh, in0=sh, scalar1=1, scalar2=None,
                                 op0=OP.bitwise_and)
                yield
                bf = ppool.tile([P, PP], F32, tag="pA")
                SC.activation(out=bf, in_=sh, func=AF.Copy, scale=BIG)
                yield
                VE.tensor_tensor(out=score, in0=score, in1=bf, op=OP.max)
                yield

                scorev = score.rearrange("p (r q) -> p r q", q=K2)
                smin = opool.tile([P, rc], F32, tag="smin")
                VE.tensor_reduce(out=smin, in_=scorev, axis=AX.X, op=OP.min)
                yield
                smin_q = bcv(smin.unsqueeze(2), (P, rc, K2))
                eqm = ppool.tile([P, PP], F32, tag="eqm")
                eqmv = eqm.rearrange("p (r q) -> p r q", q=K2)
                VE.tensor_tensor(out=eqmv, in0=scorev, in1=smin_q,
                                 op=OP.not_equal)
                yield
                i64_q = bcv(IOTA64.unsqueeze(1), (P, rc, K2))
                pidt = ppool.tile([P, PP], F32, tag="dot01")
                pidtv = pidt.rearrange("p (r q) -> p r q", q=K2)
                VE.scalar_tensor_tensor(out=pidtv, in0=eqmv, scalar=BIG,
                                        in1=i64_q, op0=OP.mult, op1=OP.add)
                yield
                pidx = opool.tile([P, rc], F32, tag="pidx")
                VE.tensor_reduce(out=pidx, in_=pidtv, axis=AX.X, op=OP.min)
                yield
                pidx_q = bcv(pidx.unsqueeze(2), (P, rc, K2))
                oh = ppool.tile([P, PP], F32, tag="eqm")
                ohv_ = oh.rearrange("p (r q) -> p r q", q=K2)
                VE.tensor_tensor(out=ohv_, in0=i64_q, in1=pidx_q, op=OP.is_equal)
                yield
                oh4 = oh.rearrange("p (r i j) -> p r i j", i=K0, j=K0)
                Ga = ppool.tile([P, PP], F32, tag="sq")
                Gb = ppool.tile([P, PP], F32, tag="sr")
                VE.tensor_tensor(out=Ga.rearrange("p (r q) -> p r q", q=K2),
                                 in0=ohv_, in1=w2t.rearrange(
                    "p (r q) -> p r q", q=K2), op=OP.mult)
                yield
                GP.tensor_tensor(out=Gb.rearrange("p (r i j) -> p r i j",
                                                  i=K0, j=K0),
                                 in0=oh4, in1=w1t, op=OP.mult)
                yield
                w2sel = opool.tile([P, rc], F32, tag="w2sel")
                w1sel = opool.tile([P, rc], F32, tag="w1sel")
                VE.tensor_reduce(out=w2sel, in_=Ga.rearrange(
                    "p (r q) -> p r q", q=K2), axis=AX.X, op=OP.add)
                yield
                VE.tensor_reduce(out=w1sel, in_=Gb.rearrange(
                    "p (r q) -> p r q", q=K2), axis=AX.X, op=OP.add)
                yield
                w0sel = opool.tile([P, rc], F32, tag="w0sel")
                VE.tensor_tensor(out=w0sel, in0=w2sel, in1=w1sel, op=OP.add)
                yield
                SC.activation(out=w0sel, in_=w0sel, func=AF.Copy, bias=1.0,
                              scale=-1.0)
                yield
                pidxi = opool.tile([P, rc], I32, tag="pidxi")
                i_i = opool.tile([P, rc], I32, tag="i_i")
                j_i = opool.tile([P, rc], I32, tag="j_i")
                i_f = opool.tile([P, rc], F32, tag="i_f")
                j_f = opool.tile([P, rc], F32, tag="j_f")
                VE.tensor_copy(out=pidxi, in_=pidx)
                VE.tensor_scalar(out=i_i, in0=pidxi, scalar1=3, scalar2=None,
                                 op0=OP.arith_shift_right)
                VE.tensor_scalar(out=j_i, in0=pidxi, scalar1=7, scalar2=None,
                                 op0=OP.bitwise_and)
                yield
                VE.tensor_copy(out=i_f, in_=i_i)
                VE.tensor_copy(out=j_f, in_=j_i)
                yield
                fb = opool.tile([P, rc], F32, tag="fb")
                nfb = opool.tile([P, rc], F32, tag="nfb")
                VE.tensor_scalar(out=fb, in0=smin, scalar1=1.0e38, scalar2=None,
                                 op0=OP.is_ge)
                VE.tensor_scalar(out=nfb, in0=fb, scalar1=-1.0, scalar2=1.0,
                                 op0=OP.mult, op1=OP.add)
                yield
                VE.tensor_tensor(out=w0sel, in0=w0sel, in1=nfb, op=OP.mult)
                VE.tensor_tensor(out=w2sel, in0=w2sel, in1=nfb, op=OP.mult)
                VE.tensor_tensor(out=w1sel, in0=w1sel, in1=nfb, op=OP.mult)
                yield
                VE.copy_predicated(out=i_f, mask=fb.bitcast(I32), data=o1_f)
                VE.copy_predicated(out=j_f, mask=fb.bitcast(I32), data=o1_f)
                yield
                wout = opool.tile([P, rc * 3], F32, tag="wout")
                iout = opool.tile([P, rc * 3], F32, tag="iout")
                woutv = wout.rearrange("p (r c) -> p r c", c=3)
                ioutv = iout.rearrange("p (r c) -> p r c", c=3)
                SC.copy(out=woutv[:, :, 0], in_=w0sel)
                SC.copy(out=woutv[:, :, 1], in_=w2sel)
                SC.copy(out=woutv[:, :, 2], in_=w1sel)
                yield
                SC.copy(out=ioutv[:, :, 0], in_=c_f)
                SC.copy(out=ioutv[:, :, 1], in_=i_f)
                SC.copy(out=ioutv[:, :, 2], in_=j_f)
                yield
                nc.sync.dma_start(outw_d[v0_:v1_, r0:r1, :], woutv)
                nc.sync.dma_start(outi_d[v0_:v1_, r0:r1, :], ioutv)

            def run_rr(gens):
                alive = list(gens)
                while alive:
                    nxt = []
                    for g in alive:
                        try:
                            next(g)
                            nxt.append(g)
                        except StopIteration:
                            pass
                    alive = nxt

            # sliding-window round-robin: keep up to 3 independent
            # instruction streams in flight; a chunk stream is admitted only
            # once its v-tile's p0 stream has finished (p0s[vt] exists).
            work = []
            for vt in range(n_vt):
                work.append(("p0", vt + 1)) if False else None
            work = [("c", vt, ci) for vt in range(n_vt) for ci in range(n_rch)]
            p0q = [("p", vt) for vt in range(1, n_vt)]
            # merged queue: c(0,0), c(0,1), p(1), c(1,0), c(1,1), p(2), ...
            queue = []
            for vt in range(n_vt):
                queue.append(("c", vt, 0))
                queue.append(("c", vt, 1))
                if vt + 1 < n_vt:
                    queue.append(("p", vt + 1))

            def ready(item):
                return item[0] == "p" or item[1] in p0s

            def make(item):
                if item[0] == "p":
                    return p0_gen(item[1])
                return chunk_gen(item[1], item[2])

            run_rr([p0_gen(0)])
            active = []
            WINDOW = 3
            while queue or active:
                while len(active) < WINDOW and queue and ready(queue[0]):
                    active.append(make(queue.pop(0)))
                progressed = []
                for g in active:
                    try:
                        next(g)
                        progressed.append(g)
                    except StopIteration:
                        pass
                active = progressed
                if not active and queue:
                    # head not ready (p0 still queued behind?) — shouldn't
                    # happen with this queue order, but guard against stall
                    active.append(make(queue.pop(0)))

    nc.compile()
    return nc


def make_consts():
    iota8 = np.arange(K0, dtype=np.float32).reshape(1, K0)
    iota64 = np.arange(K2, dtype=np.float32).reshape(1, K2)
    pow2 = (2.0 ** np.arange(K0)).astype(np.float32).reshape(1, K0)
    return {"iota8": iota8, "iota64": iota64, "pow2": pow2}


def make_in_maps(template, projections):
    template = np.ascontiguousarray(np.asarray(template, np.float32))
    projections = np.ascontiguousarray(np.asarray(projections, np.float32))
    consts = make_consts()
    tmplT = np.stack([template[..., 0].reshape(-1), template[..., 1].reshape(-1)])
    px_all = np.ascontiguousarray(projections[..., 0])
    py_all = np.ascontiguousarray(projections[..., 1])
    in_maps = []
    for c in range(N_CORES):
        pxc = px_all[c * VS:(c + 1) * VS]
        pyc = py_all[c * VS:(c + 1) * VS]
        pad = VSP - VS
        pxc = np.concatenate([pxc, np.broadcast_to(pxc[:1], (pad, K0))], 0)
        pyc = np.concatenate([pyc, np.broadcast_to(pyc[:1], (pad, K0))], 0)
        m = {"px": np.ascontiguousarray(pxc), "py": np.ascontiguousarray(pyc),
             "tmpl": tmplT}
        m.update(consts)
        in_maps.append(m)
    return in_maps


_NC_CACHE = {}


def kernel(template, projections, _want_time=False):
    from concourse.bass_utils import run_bass_kernel_spmd
    if "nc" not in _NC_CACHE:
        _NC_CACHE["nc"] = build_nc()
    nc = _NC_CACHE["nc"]
    in_maps = make_in_maps(template, projections)
    res = run_bass_kernel_spmd(nc, in_maps, core_ids=list(range(N_CORES)))
    ws, idxs = [], []
    for c in range(N_CORES):
        out = res.results[c]
        ws.append(out["outw"][:VS].reshape(VS, R, A, 3))
        idxs.append(out["outi"][:VS].reshape(VS, R, A, 3))
    w = np.concatenate(ws, 0).astype(np.float32)
    idx = np.rint(np.concatenate(idxs, 0)).astype(np.int32)
    if _want_time:
        return (w, idx), res
    return w, idx


# revision 14
# speedup vs baseline: 1.4598x; 1.4598x over previous
"""Trainium2 Bass kernel for BarycentricCoordinates (retrieval_knn).

v2's per-c Delaunay restructure + manual software pipelining: the two
r-chunks of a v-tile and the NEXT v-tile's per-c precompute are issued
round-robin, one instruction per slot, so consecutive ops on each engine
come from independent chains (hides DVE pipeline latency that serialized
v2 on HW). Arithmetic is identical to v2 (bitwise-matches the reference).
"""

import sys

sys.path.insert(0, "/opt/trn_rl_repo")

import numpy as np

import concourse.bass as bass
import concourse.bacc as bacc
import concourse.mybir as mybir
from concourse.tile import TileContext

F32 = mybir.dt.float32
I32 = mybir.dt.int32
OP = mybir.AluOpType
AF = mybir.ActivationFunctionType
AX = mybir.AxisListType

BIG = 2.0e38
N_CORES = 8
V_TOTAL = 5000
R, A, K0 = 5, 8, 8
RA = R * A
VS = V_TOTAL // N_CORES
P = 128
VSP = 640
RC = 20
K2 = 64
K3 = 512


def build_nc(vsp=VSP, rc=RC, ra=RA):
    nc = bacc.Bacc("TRN2", target_bir_lowering=False)
    n_vt = vsp // P
    n_rch = ra // rc

    px_d = nc.dram_tensor("px", (vsp, K0), F32, kind="ExternalInput")
    py_d = nc.dram_tensor("py", (vsp, K0), F32, kind="ExternalInput")
    tmpl_d = nc.dram_tensor("tmpl", (2, ra), F32, kind="ExternalInput")
    iota8_d = nc.dram_tensor("iota8", (1, K0), F32, kind="ExternalInput")
    iota64_d = nc.dram_tensor("iota64", (1, K2), F32, kind="ExternalInput")
    pow2_d = nc.dram_tensor("pow2", (1, K0), F32, kind="ExternalInput")
    outw_d = nc.dram_tensor("outw", (vsp, ra, 3), F32, kind="ExternalOutput")
    outi_d = nc.dram_tensor("outi", (vsp, ra, 3), F32, kind="ExternalOutput")

    with TileContext(nc) as tc:
        VE = nc.vector
        GP = nc.gpsimd
        SC = nc.scalar
        PP = rc * K2
        RK = rc * K0

        with (
            tc.tile_pool(name="const", bufs=1) as cpool,
            tc.tile_pool(name="vt", bufs=2) as vpool,
            tc.tile_pool(name="det", bufs=1) as spool,
            tc.tile_pool(name="pair", bufs=2) as ppool,
            tc.tile_pool(name="rk", bufs=2) as rkpool,
            tc.tile_pool(name="small", bufs=2) as opool,
        ):
            TX = cpool.tile([P, ra], F32, tag="TX")
            TY = cpool.tile([P, ra], F32, tag="TY")
            IOTA8 = cpool.tile([P, K0], F32, tag="IOTA8")
            IOTA64 = cpool.tile([P, K2], F32, tag="IOTA64")
            POW2 = cpool.tile([P, K0], F32, tag="POW2")
            nc.sync.dma_start(TX, tmpl_d[0:1, :].to_broadcast((P, ra)))
            nc.sync.dma_start(TY, tmpl_d[1:2, :].to_broadcast((P, ra)))
            nc.sync.dma_start(IOTA8, iota8_d[0:1, :].to_broadcast((P, K0)))
            nc.sync.dma_start(IOTA64, iota64_d[0:1, :].to_broadcast((P, K2)))
            nc.sync.dma_start(POW2, pow2_d[0:1, :].to_broadcast((P, K0)))

            def bcv(ap, shape):
                return ap.to_broadcast(shape)

            p0s = {}

            def p0_gen(vt):
                """Per-v-tile precompute: s, b-tensors, U, det(c,ij,k), bit-pack."""
                v0_, v1_ = vt * P, (vt + 1) * P
                px = vpool.tile([P, K0], F32, tag="px")
                py = vpool.tile([P, K0], F32, tag="py")
                nc.sync.dma_start(px, px_d[v0_:v1_, :])
                nc.sync.dma_start(py, py_d[v0_:v1_, :])
                yield
                s_ = vpool.tile([P, K0], F32, tag="s")
                t8 = vpool.tile([P, K0], F32, tag="t8")
                SC.activation(out=s_, in_=px, func=AF.Square)
                SC.activation(out=t8, in_=py, func=AF.Square)
                yield
                VE.tensor_tensor(out=s_, in0=s_, in1=t8, op=OP.add)
                yield
                PXYS = vpool.tile([P, 24], F32, tag="PXYS")
                SC.copy(out=PXYS[:, 0:8], in_=px)
                SC.copy(out=PXYS[:, 8:16], in_=py)
                SC.copy(out=PXYS[:, 16:24], in_=IOTA8)
                yield
                bx = vpool.tile([P, K2], F32, tag="bx")
                by = vpool.tile([P, K2], F32, tag="by")
                bs = vpool.tile([P, K2], F32, tag="bs")
                bxv = bx.rearrange("p (i k) -> p i k", k=K0)
                byv = by.rearrange("p (i k) -> p i k", k=K0)
                bsv = bs.rearrange("p (i k) -> p i k", k=K0)
                VE.tensor_tensor(out=bxv, in0=bcv(px.unsqueeze(2), (P, K0, K0)),
                                 in1=bcv(px.unsqueeze(1), (P, K0, K0)),
                                 op=OP.subtract)
                yield
                GP.tensor_tensor(out=byv, in0=bcv(py.unsqueeze(2), (P, K0, K0)),
                                 in1=bcv(py.unsqueeze(1), (P, K0, K0)),
                                 op=OP.subtract)
                yield
                VE.tensor_tensor(out=bsv, in0=bcv(s_.unsqueeze(2), (P, K0, K0)),
                                 in1=bcv(s_.unsqueeze(1), (P, K0, K0)),
                                 op=OP.subtract)
                yield

                def Bi(t):
                    return bcv(t.rearrange("p (i k) -> p i k", k=K0).unsqueeze(2),
                               (P, K0, K0, K0))

                def Bj(t):
                    return bcv(t.rearrange("p (j k) -> p j k", k=K0).unsqueeze(1),
                               (P, K0, K0, K0))

                U1 = vpool.tile([P, K3], F32, tag="U1")
                U2 = vpool.tile([P, K3], F32, tag="U2")
                U3 = vpool.tile([P, K3], F32, tag="U3")
                uA = vpool.tile([P, K3], F32, tag="uA")
                U1v = U1.rearrange("p (i j k) -> p i j k", j=K0, k=K0)
                U2v = U2.rearrange("p (i j k) -> p i j k", j=K0, k=K0)
                U3v = U3.rearrange("p (i j k) -> p i j k", j=K0, k=K0)
                uAv = uA.rearrange("p (i j k) -> p i j k", j=K0, k=K0)
                VE.tensor_tensor(out=U1v, in0=Bi(by), in1=Bj(bs), op=OP.mult)
                yield
                GP.tensor_tensor(out=uAv, in0=Bi(bs), in1=Bj(by), op=OP.mult)
                yield
                VE.tensor_tensor(out=U1, in0=U1, in1=uA, op=OP.subtract)
                yield
                VE.tensor_tensor(out=U2v, in0=Bi(bx), in1=Bj(bs), op=OP.mult)
                yield
                GP.tensor_tensor(out=uAv, in0=Bi(bs), in1=Bj(bx), op=OP.mult)
                yield
                VE.tensor_tensor(out=U2, in0=U2, in1=uA, op=OP.subtract)
                yield
                VE.tensor_tensor(out=U3v, in0=Bi(bx), in1=Bj(by), op=OP.mult)
                yield
                GP.tensor_tensor(out=uAv, in0=Bi(by), in1=Bj(bx), op=OP.mult)
                yield
                VE.tensor_tensor(out=U3, in0=U3, in1=uA, op=OP.subtract)
                yield

                det = spool.tile([P, K0 * K3], F32, tag="det")
                dtm = spool.tile([P, K0 * K3], F32, tag="dtm")
                detv = det.rearrange("p (c q k) -> p c q k", q=K2, k=K0)
                dtmv = dtm.rearrange("p (c q k) -> p c q k", q=K2, k=K0)

                def Ck(t):
                    return bcv(t.rearrange("p (c k) -> p c k", k=K0).unsqueeze(2),
                               (P, K0, K2, K0))

                def Uq(t):
                    return bcv(t.rearrange("p (q k) -> p q k", k=K0).unsqueeze(1),
                               (P, K0, K2, K0))

                VE.tensor_tensor(out=detv, in0=Ck(bx), in1=Uq(U1), op=OP.mult)
                yield
                GP.tensor_tensor(out=dtmv, in0=Ck(by), in1=Uq(U2), op=OP.mult)
                yield
                VE.tensor_tensor(out=det, in0=det, in1=dtm, op=OP.subtract)
                yield
                VE.tensor_tensor(out=dtmv, in0=Ck(bs), in1=Uq(U3), op=OP.mult)
                yield
                VE.tensor_tensor(out=det, in0=det, in1=dtm, op=OP.add)
                yield
                VD = vpool.tile([P, K3], F32, tag="VD")
                VDv = VD.rearrange("p (c q) -> p c q", q=K2)
                VE.tensor_reduce(out=VDv, in_=detv, axis=AX.X, op=OP.max)
                yield
                mpb = vpool.tile([P, K3], F32, tag="uA")
                mpbv = mpb.rearrange("p (c q) -> p c q", q=K2)
                VE.scalar_tensor_tensor(out=mpbv, in0=VDv, scalar=0.0,
                                        in1=bcv(POW2.unsqueeze(2), (P, K0, K2)),
                                        op0=OP.is_gt, op1=OP.mult)
                yield
                Mp = vpool.tile([P, K2], F32, tag="Mp")
                VE.tensor_reduce(out=Mp,
                                 in_=mpb.rearrange("p (c q) -> p q c", q=K2),
                                 axis=AX.X, op=OP.add)
                yield
                ipack = vpool.tile([P, K2], I32, tag="ipack")
                VE.tensor_copy(out=ipack, in_=Mp)
                yield
                p0s[vt] = dict(px=px, py=py, PXYS=PXYS, ipack=ipack)

            def chunk_gen(vt, rchunk):
                """Per-(v-tile, r-chunk): closest, weights, score, selection."""
                v0_, v1_ = vt * P, (vt + 1) * P
                S = p0s[vt]
                px, py, PXYS, ipack = S["px"], S["py"], S["PXYS"], S["ipack"]
                r0 = rchunk * rc
                r1 = r0 + rc

                d2 = rkpool.tile([P, RK], F32, tag="d2")
                tdx = rkpool.tile([P, RK], F32, tag="tdx")
                tdy = rkpool.tile([P, RK], F32, tag="tdy")
                d2v = d2.rearrange("p (r k) -> p r k", k=K0)
                tdxv = tdx.rearrange("p (r k) -> p r k", k=K0)
                tdyv = tdy.rearrange("p (r k) -> p r k", k=K0)
                px_rk = bcv(px.unsqueeze(1), (P, rc, K0))
                py_rk = bcv(py.unsqueeze(1), (P, rc, K0))
                tx_rk = bcv(TX[:, r0:r1].unsqueeze(2), (P, rc, K0))
                ty_rk = bcv(TY[:, r0:r1].unsqueeze(2), (P, rc, K0))
                VE.tensor_tensor(out=tdxv, in0=px_rk, in1=tx_rk, op=OP.subtract)
                yield
                GP.tensor_tensor(out=tdyv, in0=py_rk, in1=ty_rk, op=OP.subtract)
                yield
                SC.activation(out=tdx, in_=tdx, func=AF.Square)
                yield
                SC.activation(out=tdy, in_=tdy, func=AF.Square)
                yield
                VE.tensor_tensor(out=d2, in0=tdx, in1=tdy, op=OP.add)
                yield
                dmin = opool.tile([P, rc], F32, tag="dmin")
                VE.tensor_reduce(out=dmin, in_=d2v, axis=AX.X, op=OP.min)
                yield
                dmin_rk = bcv(dmin.unsqueeze(2), (P, rc, K0))
                m0 = rkpool.tile([P, RK], F32, tag="m0")
                m0v = m0.rearrange("p (r k) -> p r k", k=K0)
                VE.tensor_tensor(out=m0v, in0=d2v, in1=dmin_rk, op=OP.is_equal)
                yield
                tA3 = rkpool.tile([P, RK * 3], F32, tag="tA3")
                tA3v = tA3.rearrange("p (r g k) -> p r g k", g=3, k=K0)
                tA3r = rkpool.tile([P, rc * 3], F32, tag="tA3r")
                tA3rv = tA3r.rearrange("p (r g) -> p r g", g=3)
                m0_rgk = bcv(m0v.unsqueeze(2), (P, rc, 3, K0))
                pxys_rgk = bcv(PXYS.rearrange("p (g k) -> p g k", k=K0)
                               .unsqueeze(1), (P, rc, 3, K0))
                GP.tensor_tensor(out=tA3v, in0=m0_rgk, in1=pxys_rgk, op=OP.mult)
                yield
                VE.tensor_reduce(out=tA3rv, in_=tA3v, axis=AX.X, op=OP.add)
                yield
                cx = tA3rv[:, :, 0:1].squeeze(2)
                cy = tA3rv[:, :, 1:2].squeeze(2)
                c_f = tA3rv[:, :, 2:3].squeeze(2)

                d2b = rkpool.tile([P, RK], F32, tag="d2b")
                VE.scalar_tensor_tensor(out=d2b, in0=m0, scalar=BIG, in1=d2,
                                        op0=OP.mult, op1=OP.add)
                yield
                dmin2 = opool.tile([P, rc], F32, tag="dmin2")
                d2bv = d2b.rearrange("p (r k) -> p r k", k=K0)
                VE.tensor_reduce(out=dmin2, in_=d2bv, axis=AX.X, op=OP.min)
                yield
                dmin2_rk = bcv(dmin2.unsqueeze(2), (P, rc, K0))
                tA = rkpool.tile([P, RK], F32, tag="tA")
                tAv = tA.rearrange("p (r k) -> p r k", k=K0)
                i8_rk = bcv(IOTA8.unsqueeze(1), (P, rc, K0))
                VE.tensor_tensor(out=tAv, in0=d2bv, in1=dmin2_rk, op=OP.is_equal)
                yield
                GP.tensor_tensor(out=tAv, in0=tAv, in1=i8_rk, op=OP.mult)
                yield
                o1_f = opool.tile([P, rc], F32, tag="o1_f")
                VE.tensor_reduce(out=o1_f, in_=tAv, axis=AX.X, op=OP.add)
                yield

                v0x = rkpool.tile([P, RK], F32, tag="v0x")
                v0y = rkpool.tile([P, RK], F32, tag="v0y")
                d00 = rkpool.tile([P, RK], F32, tag="d00")
                d02 = rkpool.tile([P, RK], F32, tag="d02")
                tB = rkpool.tile([P, RK], F32, tag="tB")
                v0xv = v0x.rearrange("p (r k) -> p r k", k=K0)
                v0yv = v0y.rearrange("p (r k) -> p r k", k=K0)
                d02v = d02.rearrange("p (r k) -> p r k", k=K0)
                tBv = tB.rearrange("p (r k) -> p r k", k=K0)
                cx_rk = bcv(cx.unsqueeze(2), (P, rc, K0))
                cy_rk = bcv(cy.unsqueeze(2), (P, rc, K0))
                VE.tensor_tensor(out=v0xv, in0=px_rk, in1=cx_rk, op=OP.subtract)
                yield
                GP.tensor_tensor(out=v0yv, in0=py_rk, in1=cy_rk, op=OP.subtract)
                yield
                v2x = opool.tile([P, rc], F32, tag="v2x")
                v2y = opool.tile([P, rc], F32, tag="v2y")
                VE.tensor_tensor(out=v2x, in0=TX[:, r0:r1], in1=cx, op=OP.subtract)
                VE.tensor_tensor(out=v2y, in0=TY[:, r0:r1], in1=cy, op=OP.subtract)
                yield
                SC.activation(out=d00, in_=v0x, func=AF.Square)
                yield
                SC.activation(out=tB, in_=v0y, func=AF.Square)
                yield
                VE.tensor_tensor(out=d00, in0=d00, in1=tB, op=OP.add)
                yield
                v2x_rk = bcv(v2x.unsqueeze(2), (P, rc, K0))
                v2y_rk = bcv(v2y.unsqueeze(2), (P, rc, K0))
                VE.tensor_tensor(out=d02v, in0=v0xv, in1=v2x_rk, op=OP.mult)
                yield
                GP.tensor_tensor(out=tBv, in0=v0yv, in1=v2y_rk, op=OP.mult)
                yield
                VE.tensor_tensor(out=d02, in0=d02, in1=tB, op=OP.add)
                yield

                def XI(t2):
                    return bcv(t2.rearrange("p (r k) -> p r k", k=K0).unsqueeze(3),
                               (P, rc, K0, K0))

                def XJ(t2):
                    return bcv(t2.rearrange("p (r k) -> p r k", k=K0).unsqueeze(2),
                               (P, rc, K0, K0))

                dot01 = ppool.tile([P, PP], F32, tag="dot01")
                pA = ppool.tile([P, PP], F32, tag="pA")
                pB = ppool.tile([P, PP], F32, tag="pB")
                w2t = ppool.tile([P, PP], F32, tag="w2t")
                w0t = ppool.tile([P, PP], F32, tag="w0t")
                inv = ppool.tile([P, PP], F32, tag="inv")
                dot01v = dot01.rearrange("p (r i j) -> p r i j", i=K0, j=K0)
                pAv = pA.rearrange("p (r i j) -> p r i j", i=K0, j=K0)
                pBv = pB.rearrange("p (r i j) -> p r i j", i=K0, j=K0)

                VE.tensor_tensor(out=dot01v, in0=XI(v0x), in1=XJ(v0x), op=OP.mult)
                yield
                GP.tensor_tensor(out=pAv, in0=XI(v0y), in1=XJ(v0y), op=OP.mult)
                yield
                VE.tensor_tensor(out=dot01, in0=dot01, in1=pA, op=OP.add)
                yield
                GP.tensor_tensor(out=pAv, in0=XI(d00), in1=XJ(d00), op=OP.mult)
                yield
                SC.activation(out=pB, in_=dot01, func=AF.Square)
                yield
                VE.tensor_tensor(out=pA, in0=pA, in1=pB, op=OP.subtract)  # denom
                yield
                VE.reciprocal(out=inv, in_=pA)
                yield
                VE.tensor_scalar(out=inv, in0=inv, scalar1=BIG, scalar2=-BIG,
                                 op0=OP.min, op1=OP.max)
                yield
                VE.tensor_tensor(out=pAv, in0=XJ(d00), in1=XI(d02), op=OP.mult)
                yield
                GP.tensor_tensor(out=pBv, in0=dot01v, in1=XJ(d02), op=OP.mult)
                yield
                VE.tensor_tensor(out=w2t, in0=pA, in1=pB, op=OP.subtract)
                yield
                VE.tensor_tensor(out=w2t, in0=w2t, in1=inv, op=OP.mult)
                yield
                w2tv = w2t.rearrange("p (r i j) -> p r i j", i=K0, j=K0)
                w1t = w2t.rearrange("p (r i j) -> p r j i", i=K0, j=K0)
                VE.tensor_tensor(out=pAv, in0=w2tv, in1=w1t, op=OP.add)
                yield
                SC.activation(out=w0t, in_=pA, func=AF.Copy, bias=1.0, scale=-1.0)
                yield
                wm = ppool.tile([P, PP], F32, tag="wm")
                wmv = wm.rearrange("p (r i j) -> p r i j", i=K0, j=K0)
                VE.tensor_tensor(out=wmv, in0=w1t, in1=w2tv, op=OP.min)
                yield
                VE.tensor_tensor(out=wm, in0=wm, in1=w0t, op=OP.min)
                yield
                sq = ppool.tile([P, PP], F32, tag="sq")
                sr = ppool.tile([P, PP], F32, tag="sr")
                SC.activation(out=sr, in_=w0t, func=AF.Square)
                yield
                SC.activation(out=sq, in_=w2t, func=AF.Square)
                yield
                VE.tensor_tensor(out=sr, in0=sr, in1=sq, op=OP.max)
                yield
                srv = sr.rearrange("p (r i j) -> p r i j", i=K0, j=K0)
                VE.tensor_tensor(out=srv, in0=srv,
                                 in1=sq.rearrange("p (r i j) -> p r j i",
                                                  i=K0, j=K0), op=OP.max)
                yield
                VE.tensor_scalar(out=pB, in0=wm, scalar1=0.0, scalar2=BIG,
                                 op0=OP.is_le, op1=OP.mult)
                yield
                score = ppool.tile([P, PP], F32, tag="score")
                VE.tensor_tensor(out=score, in0=sr, in1=pB, op=OP.max)
                yield
                c_i = opool.tile([P, rc], I32, tag="c_i")
                VE.tensor_copy(out=c_i, in_=c_f)
                VE.tensor_scalar(out=c_i, in0=c_i, scalar1=-1, scalar2=31,
                                 op0=OP.mult, op1=OP.add)
                yield
                shf = ppool.tile([P, PP], F32, tag="pB")
                sh = shf.bitcast(I32)
                shv = sh.rearrange("p (r q) -> p r q", q=K2)
                VE.tensor_tensor(out=shv,
                                 in0=bcv(ipack.unsqueeze(1), (P, rc, K2)),
                                 in1=bcv(c_i.unsqueeze(2), (P, rc, K2)),
                                 op=OP.logical_shift_left)
                yield
                bf = ppool.tile([P, PP], F32, tag="pA")
                VE.tensor_scalar(out=bf, in0=sh, scalar1=0, scalar2=BIG,
                                 op0=OP.is_lt, op1=OP.mult)
                yield
                VE.tensor_tensor(out=score, in0=score, in1=bf, op=OP.max)
                yield

                scorev = score.rearrange("p (r q) -> p r q", q=K2)
                smin = opool.tile([P, rc], F32, tag="smin")
                VE.tensor_reduce(out=smin, in_=scorev, axis=AX.X, op=OP.min)
                yield
                smin_q = bcv(smin.unsqueeze(2), (P, rc, K2))
                eqm = ppool.tile([P, PP], F32, tag="eqm")
                eqmv = eqm.rearrange("p (r q) -> p r q", q=K2)
                VE.tensor_tensor(out=eqmv, in0=scorev, in1=smin_q,
                                 op=OP.not_equal)
                yield
                i64_q = bcv(IOTA64.unsqueeze(1), (P, rc, K2))
                pidt = ppool.tile([P, PP], F32, tag="dot01")
                pidtv = pidt.rearrange("p (r q) -> p r q", q=K2)
                VE.scalar_tensor_tensor(out=pidtv, in0=eqmv, scalar=BIG,
                                        in1=i64_q, op0=OP.mult, op1=OP.add)
                yield
                pidx = opool.tile([P, rc], F32, tag="pidx")
                VE.tensor_reduce(out=pidx, in_=pidtv, axis=AX.X, op=OP.min)
                yield
                pidx_q = bcv(pidx.unsqueeze(2), (P, rc, K2))
                oh = ppool.tile([P, PP], F32, tag="eqm")
                ohv_ = oh.rearrange("p (r q) -> p r q", q=K2)
                VE.tensor_tensor(out=ohv_, in0=i64_q, in1=pidx_q, op=OP.is_equal)
                yield
                oh4 = oh.rearrange("p (r i j) -> p r i j", i=K0, j=K0)
                Ga = ppool.tile([P, PP], F32, tag="sq")
                Gb = ppool.tile([P, PP], F32, tag="sr")
                VE.tensor_tensor(out=Ga.rearrange("p (r q) -> p r q", q=K2),
                                 in0=ohv_, in1=w2t.rearrange(
                    "p (r q) -> p r q", q=K2), op=OP.mult)
                yield
                GP.tensor_tensor(out=Gb.rearrange("p (r i j) -> p r i j",
                                                  i=K0, j=K0),
                                 in0=oh4, in1=w1t, op=OP.mult)
                yield
                wout = opool.tile([P, rc * 3], F32, tag="wout")
                woutv = wout.rearrange("p (r c) -> p r c", c=3)
                w2sel = woutv[:, :, 1]
                w1sel = woutv[:, :, 2]
                VE.tensor_reduce(out=w2sel, in_=Ga.rearrange(
                    "p (r q) -> p r q", q=K2), axis=AX.X, op=OP.add)
                yield
                VE.tensor_reduce(out=w1sel, in_=Gb.rearrange(
                    "p (r q) -> p r q", q=K2), axis=AX.X, op=OP.add)
                yield
                w0sel = woutv[:, :, 0]
                VE.tensor_tensor(out=w0sel, in0=w2sel, in1=w1sel, op=OP.add)
                yield
                SC.activation(out=w0sel, in_=w0sel, func=AF.Copy, bias=1.0,
                              scale=-1.0)
                yield
                pidxi = opool.tile([P, rc], I32, tag="pidxi")
                i_i = opool.tile([P, rc], I32, tag="i_i")
                j_i = opool.tile([P, rc], I32, tag="j_i")
                i_f = opool.tile([P, rc], F32, tag="i_f")
                j_f = opool.tile([P, rc], F32, tag="j_f")
                VE.tensor_copy(out=pidxi, in_=pidx)
                VE.tensor_scalar(out=i_i, in0=pidxi, scalar1=3, scalar2=None,
                                 op0=OP.arith_shift_right)
                VE.tensor_scalar(out=j_i, in0=pidxi, scalar1=7, scalar2=None,
                                 op0=OP.bitwise_and)
                yield
                VE.tensor_copy(out=i_f, in_=i_i)
                VE.tensor_copy(out=j_f, in_=j_i)
                yield
                fb = opool.tile([P, rc], F32, tag="fb")
                nfb = opool.tile([P, rc], F32, tag="nfb")
                VE.tensor_scalar(out=fb, in0=smin, scalar1=1.0e38, scalar2=None,
                                 op0=OP.is_ge)
                VE.tensor_scalar(out=nfb, in0=fb, scalar1=-1.0, scalar2=1.0,
                                 op0=OP.mult, op1=OP.add)
                yield
                VE.tensor_tensor(out=w0sel, in0=w0sel, in1=nfb, op=OP.mult)
                VE.tensor_tensor(out=w2sel, in0=w2sel, in1=nfb, op=OP.mult)
                VE.tensor_tensor(out=w1sel, in0=w1sel, in1=nfb, op=OP.mult)
                yield
                VE.copy_predicated(out=i_f, mask=fb.bitcast(I32), data=o1_f)
                VE.copy_predicated(out=j_f, mask=fb.bitcast(I32), data=o1_f)
                yield
                iout = opool.tile([P, rc * 3], F32, tag="iout")
                ioutv = iout.rearrange("p (r c) -> p r c", c=3)
                SC.copy(out=ioutv[:, :, 0], in_=c_f)
                SC.copy(out=ioutv[:, :, 1], in_=i_f)
                SC.copy(out=ioutv[:, :, 2], in_=j_f)
                yield
                nc.sync.dma_start(outw_d[v0_:v1_, r0:r1, :], woutv)
                nc.sync.dma_start(outi_d[v0_:v1_, r0:r1, :], ioutv)

            def run_rr(gens):
                alive = list(gens)
                while alive:
                    nxt = []
                    for g in alive:
                        try:
                            next(g)
                            nxt.append(g)
                        except StopIteration:
                            pass
                    alive = nxt

            # sliding-window round-robin: keep up to 3 independent
            # instruction streams in flight; a chunk stream is admitted only
            # once its v-tile's p0 stream has finished (p0s[vt] exists).
            work = []
            for vt in range(n_vt):
                work.append(("p0", vt + 1)) if False else None
            work = [("c", vt, ci) for vt in range(n_vt) for ci in range(n_rch)]
            p0q = [("p", vt) for vt in range(1, n_vt)]
            # merged queue: c(0,0), c(0,1), p(1), c(1,0), c(1,1), p(2), ...
            queue = []
            for vt in range(n_vt):
                queue.append(("c", vt, 0))
                queue.append(("c", vt, 1))
                if vt + 1 < n_vt:
                    queue.append(("p", vt + 1))

            def ready(item):
                return item[0] == "p" or item[1] in p0s

            def make(item):
                if item[0] == "p":
                    return p0_gen(item[1])
                return chunk_gen(item[1], item[2])

            run_rr([p0_gen(0)])
            active = []
            WINDOW = 3
            while queue or active:
                while len(active) < WINDOW and queue and ready(queue[0]):
                    active.append(make(queue.pop(0)))
                progressed = []
                for g in active:
                    try:
                        next(g)
                        progressed.append(g)
                    except StopIteration:
                        pass
                active = progressed
                if not active and queue:
                    # head not ready (p0 still queued behind?) — shouldn't
                    # happen with this queue order, but guard against stall
                    active.append(make(queue.pop(0)))

    nc.compile()
    return nc


def make_consts():
    iota8 = np.arange(K0, dtype=np.float32).reshape(1, K0)
    iota64 = np.arange(K2, dtype=np.float32).reshape(1, K2)
    pow2 = (2.0 ** np.arange(K0)).astype(np.float32).reshape(1, K0)
    return {"iota8": iota8, "iota64": iota64, "pow2": pow2}


def make_in_maps(template, projections):
    template = np.ascontiguousarray(np.asarray(template, np.float32))
    projections = np.ascontiguousarray(np.asarray(projections, np.float32))
    consts = make_consts()
    tmplT = np.stack([template[..., 0].reshape(-1), template[..., 1].reshape(-1)])
    px_all = np.ascontiguousarray(projections[..., 0])
    py_all = np.ascontiguousarray(projections[..., 1])
    in_maps = []
    for c in range(N_CORES):
        pxc = px_all[c * VS:(c + 1) * VS]
        pyc = py_all[c * VS:(c + 1) * VS]
        pad = VSP - VS
        pxc = np.concatenate([pxc, np.broadcast_to(pxc[:1], (pad, K0))], 0)
        pyc = np.concatenate([pyc, np.broadcast_to(pyc[:1], (pad, K0))], 0)
        m = {"px": np.ascontiguousarray(pxc), "py": np.ascontiguousarray(pyc),
             "tmpl": tmplT}
        m.update(consts)
        in_maps.append(m)
    return in_maps


_NC_CACHE = {}


def kernel(template, projections, _want_time=False):
    from concourse.bass_utils import run_bass_kernel_spmd
    if "nc" not in _NC_CACHE:
        _NC_CACHE["nc"] = build_nc()
    nc = _NC_CACHE["nc"]
    in_maps = make_in_maps(template, projections)
    res = run_bass_kernel_spmd(nc, in_maps, core_ids=list(range(N_CORES)))
    ws, idxs = [], []
    for c in range(N_CORES):
        out = res.results[c]
        ws.append(out["outw"][:VS].reshape(VS, R, A, 3))
        idxs.append(out["outi"][:VS].reshape(VS, R, A, 3))
    w = np.concatenate(ws, 0).astype(np.float32)
    idx = np.rint(np.concatenate(idxs, 0)).astype(np.int32)
    if _want_time:
        return (w, idx), res
    return w, idx


# revision 15
# speedup vs baseline: 1.8278x; 1.2521x over previous
"""Trainium2 Bass kernel for BarycentricCoordinates (retrieval_knn).

v2's per-c Delaunay restructure + manual software pipelining: the two
r-chunks of a v-tile and the NEXT v-tile's per-c precompute are issued
round-robin, one instruction per slot, so consecutive ops on each engine
come from independent chains (hides DVE pipeline latency that serialized
v2 on HW). Arithmetic is identical to v2 (bitwise-matches the reference).
"""

import sys

sys.path.insert(0, "/opt/trn_rl_repo")

import numpy as np

import concourse.bass as bass
import concourse.bacc as bacc
import concourse.mybir as mybir
from concourse.tile import TileContext

F32 = mybir.dt.float32
I32 = mybir.dt.int32
OP = mybir.AluOpType
AF = mybir.ActivationFunctionType
AX = mybir.AxisListType

BIG = 2.0e38
N_CORES = 8
V_TOTAL = 5000
R, A, K0 = 5, 8, 8
RA = R * A
VS = V_TOTAL // N_CORES
P = 128
VSP = 640
RC = 20
K2 = 64
K3 = 512


def build_nc(vsp=VSP, rc=RC, ra=RA):
    nc = bacc.Bacc("TRN2", target_bir_lowering=False)
    n_vt = vsp // P
    n_rch = ra // rc

    px_d = nc.dram_tensor("px", (vsp, K0), F32, kind="ExternalInput")
    py_d = nc.dram_tensor("py", (vsp, K0), F32, kind="ExternalInput")
    tmpl_d = nc.dram_tensor("tmpl", (2, ra), F32, kind="ExternalInput")
    iota8_d = nc.dram_tensor("iota8", (1, K0), F32, kind="ExternalInput")
    iota64_d = nc.dram_tensor("iota64", (1, K2), F32, kind="ExternalInput")
    iota64b_d = nc.dram_tensor("iota64b", (1, K2), F32, kind="ExternalInput")
    pow2_d = nc.dram_tensor("pow2", (1, K0), F32, kind="ExternalInput")
    outw_d = nc.dram_tensor("outw", (vsp, ra, 3), F32, kind="ExternalOutput")
    outi_d = nc.dram_tensor("outi", (vsp, ra, 3), F32, kind="ExternalOutput")

    with TileContext(nc) as tc:
        VE = nc.vector
        GP = nc.gpsimd
        SC = nc.scalar
        PP = rc * K2
        RK = rc * K0

        with (
            tc.tile_pool(name="const", bufs=1) as cpool,
            tc.tile_pool(name="vt", bufs=2) as vpool,
            tc.tile_pool(name="det", bufs=1) as spool,
            tc.tile_pool(name="pair", bufs=2) as ppool,
            tc.tile_pool(name="rk", bufs=2) as rkpool,
            tc.tile_pool(name="small", bufs=2) as opool,
        ):
            TX = cpool.tile([P, ra], F32, tag="TX")
            TY = cpool.tile([P, ra], F32, tag="TY")
            IOTA8 = cpool.tile([P, K0], F32, tag="IOTA8")
            IOTA64 = cpool.tile([P, K2], F32, tag="IOTA64")
            IOTA64B = cpool.tile([P, K2], F32, tag="IOTA64B")
            POW2 = cpool.tile([P, K0], F32, tag="POW2")
            nc.sync.dma_start(TX, tmpl_d[0:1, :].to_broadcast((P, ra)))
            nc.sync.dma_start(TY, tmpl_d[1:2, :].to_broadcast((P, ra)))
            nc.sync.dma_start(IOTA8, iota8_d[0:1, :].to_broadcast((P, K0)))
            nc.sync.dma_start(IOTA64, iota64_d[0:1, :].to_broadcast((P, K2)))
            nc.sync.dma_start(IOTA64B, iota64b_d[0:1, :].to_broadcast((P, K2)))
            nc.sync.dma_start(POW2, pow2_d[0:1, :].to_broadcast((P, K0)))

            def bcv(ap, shape):
                return ap.to_broadcast(shape)

            p0s = {}

            def p0_gen(vt):
                """Per-v-tile precompute: s, b-tensors, U, det(c,ij,k), bit-pack."""
                v0_, v1_ = vt * P, (vt + 1) * P
                px = vpool.tile([P, K0], F32, tag="px")
                py = vpool.tile([P, K0], F32, tag="py")
                nc.sync.dma_start(px, px_d[v0_:v1_, :])
                nc.sync.dma_start(py, py_d[v0_:v1_, :])
                yield
                s_ = vpool.tile([P, K0], F32, tag="s")
                t8 = vpool.tile([P, K0], F32, tag="t8")
                SC.activation(out=s_, in_=px, func=AF.Square)
                SC.activation(out=t8, in_=py, func=AF.Square)
                yield
                VE.tensor_tensor(out=s_, in0=s_, in1=t8, op=OP.add)
                yield
                PXYS = vpool.tile([P, 24], F32, tag="PXYS")
                SC.copy(out=PXYS[:, 0:8], in_=px)
                SC.copy(out=PXYS[:, 8:16], in_=py)
                SC.copy(out=PXYS[:, 16:24], in_=IOTA8)
                yield
                bx = vpool.tile([P, K2], F32, tag="bx")
                by = vpool.tile([P, K2], F32, tag="by")
                bs = vpool.tile([P, K2], F32, tag="bs")
                bxv = bx.rearrange("p (i k) -> p i k", k=K0)
                byv = by.rearrange("p (i k) -> p i k", k=K0)
                bsv = bs.rearrange("p (i k) -> p i k", k=K0)
                VE.tensor_tensor(out=bxv, in0=bcv(px.unsqueeze(2), (P, K0, K0)),
                                 in1=bcv(px.unsqueeze(1), (P, K0, K0)),
                                 op=OP.subtract)
                yield
                GP.tensor_tensor(out=byv, in0=bcv(py.unsqueeze(2), (P, K0, K0)),
                                 in1=bcv(py.unsqueeze(1), (P, K0, K0)),
                                 op=OP.subtract)
                yield
                VE.tensor_tensor(out=bsv, in0=bcv(s_.unsqueeze(2), (P, K0, K0)),
                                 in1=bcv(s_.unsqueeze(1), (P, K0, K0)),
                                 op=OP.subtract)
                yield

                def Bi(t):
                    return bcv(t.rearrange("p (i k) -> p i k", k=K0).unsqueeze(2),
                               (P, K0, K0, K0))

                def Bj(t):
                    return bcv(t.rearrange("p (j k) -> p j k", k=K0).unsqueeze(1),
                               (P, K0, K0, K0))

                U1 = vpool.tile([P, K3], F32, tag="U1")
                U2 = vpool.tile([P, K3], F32, tag="U2")
                U3 = vpool.tile([P, K3], F32, tag="U3")
                uA = vpool.tile([P, K3], F32, tag="uA")
                U1v = U1.rearrange("p (i j k) -> p i j k", j=K0, k=K0)
                U2v = U2.rearrange("p (i j k) -> p i j k", j=K0, k=K0)
                U3v = U3.rearrange("p (i j k) -> p i j k", j=K0, k=K0)
                uAv = uA.rearrange("p (i j k) -> p i j k", j=K0, k=K0)
                VE.tensor_tensor(out=U1v, in0=Bi(by), in1=Bj(bs), op=OP.mult)
                yield
                GP.tensor_tensor(out=uAv, in0=Bi(bs), in1=Bj(by), op=OP.mult)
                yield
                VE.tensor_tensor(out=U1, in0=U1, in1=uA, op=OP.subtract)
                yield
                GP.tensor_tensor(out=U2v, in0=Bi(bx), in1=Bj(bs), op=OP.mult)
                yield
                GP.tensor_tensor(out=uAv, in0=Bi(bs), in1=Bj(bx), op=OP.mult)
                yield
                VE.tensor_tensor(out=U2, in0=U2, in1=uA, op=OP.subtract)
                yield
                VE.tensor_tensor(out=U3v, in0=Bi(bx), in1=Bj(by), op=OP.mult)
                yield
                GP.tensor_tensor(out=uAv, in0=Bi(by), in1=Bj(bx), op=OP.mult)
                yield
                VE.tensor_tensor(out=U3, in0=U3, in1=uA, op=OP.subtract)
                yield

                det = spool.tile([P, K0 * K3], F32, tag="det")
                dtm = spool.tile([P, K0 * K3], F32, tag="dtm")
                detv = det.rearrange("p (c q k) -> p c q k", q=K2, k=K0)
                dtmv = dtm.rearrange("p (c q k) -> p c q k", q=K2, k=K0)

                def Ck(t):
                    return bcv(t.rearrange("p (c k) -> p c k", k=K0).unsqueeze(2),
                               (P, K0, K2, K0))

                def Uq(t):
                    return bcv(t.rearrange("p (q k) -> p q k", k=K0).unsqueeze(1),
                               (P, K0, K2, K0))

                VE.tensor_tensor(out=detv, in0=Ck(bx), in1=Uq(U1), op=OP.mult)
                yield
                GP.tensor_tensor(out=dtmv, in0=Ck(by), in1=Uq(U2), op=OP.mult)
                yield
                VE.tensor_tensor(out=det, in0=det, in1=dtm, op=OP.subtract)
                yield
                GP.tensor_tensor(out=dtmv, in0=Ck(bs), in1=Uq(U3), op=OP.mult)
                yield
                VE.tensor_tensor(out=det, in0=det, in1=dtm, op=OP.add)
                yield
                VD = vpool.tile([P, K3], F32, tag="VD")
                VDv = VD.rearrange("p (c q) -> p c q", q=K2)
                VE.tensor_reduce(out=VDv, in_=detv, axis=AX.X, op=OP.max)
                yield
                mpb = vpool.tile([P, K3], F32, tag="uA")
                mpbv = mpb.rearrange("p (c q) -> p c q", q=K2)
                VE.scalar_tensor_tensor(out=mpbv, in0=VDv, scalar=0.0,
                                        in1=bcv(POW2.unsqueeze(2), (P, K0, K2)),
                                        op0=OP.is_gt, op1=OP.mult)
                yield
                Mp = vpool.tile([P, K2], F32, tag="Mp")
                VE.tensor_reduce(out=Mp,
                                 in_=mpb.rearrange("p (c q) -> p q c", q=K2),
                                 axis=AX.X, op=OP.add)
                yield
                ipack = vpool.tile([P, K2], I32, tag="ipack")
                VE.tensor_copy(out=ipack, in_=Mp)
                yield
                p0s[vt] = dict(px=px, py=py, PXYS=PXYS, ipack=ipack)

            def chunk_gen(vt, rchunk):
                """Per-(v-tile, r-chunk): closest, weights, score, selection."""
                v0_, v1_ = vt * P, (vt + 1) * P
                S = p0s[vt]
                px, py, PXYS, ipack = S["px"], S["py"], S["PXYS"], S["ipack"]
                r0 = rchunk * rc
                r1 = r0 + rc

                d2 = rkpool.tile([P, RK], F32, tag="d2")
                tdx = rkpool.tile([P, RK], F32, tag="tdx")
                tdy = rkpool.tile([P, RK], F32, tag="tdy")
                d2v = d2.rearrange("p (r k) -> p r k", k=K0)
                tdxv = tdx.rearrange("p (r k) -> p r k", k=K0)
                tdyv = tdy.rearrange("p (r k) -> p r k", k=K0)
                px_rk = bcv(px.unsqueeze(1), (P, rc, K0))
                py_rk = bcv(py.unsqueeze(1), (P, rc, K0))
                tx_rk = bcv(TX[:, r0:r1].unsqueeze(2), (P, rc, K0))
                ty_rk = bcv(TY[:, r0:r1].unsqueeze(2), (P, rc, K0))
                VE.tensor_tensor(out=tdxv, in0=px_rk, in1=tx_rk, op=OP.subtract)
                yield
                GP.tensor_tensor(out=tdyv, in0=py_rk, in1=ty_rk, op=OP.subtract)
                yield
                SC.activation(out=tdx, in_=tdx, func=AF.Square)
                yield
                SC.activation(out=tdy, in_=tdy, func=AF.Square)
                yield
                VE.tensor_tensor(out=d2, in0=tdx, in1=tdy, op=OP.add)
                yield
                dmin = opool.tile([P, rc], F32, tag="dmin")
                VE.tensor_reduce(out=dmin, in_=d2v, axis=AX.X, op=OP.min)
                yield
                dmin_rk = bcv(dmin.unsqueeze(2), (P, rc, K0))
                m0 = rkpool.tile([P, RK], F32, tag="m0")
                m0v = m0.rearrange("p (r k) -> p r k", k=K0)
                VE.tensor_tensor(out=m0v, in0=d2v, in1=dmin_rk, op=OP.is_equal)
                yield
                tA3 = rkpool.tile([P, RK * 3], F32, tag="tA3")
                tA3v = tA3.rearrange("p (r g k) -> p r g k", g=3, k=K0)
                tA3r = rkpool.tile([P, rc * 3], F32, tag="tA3r")
                tA3rv = tA3r.rearrange("p (r g) -> p r g", g=3)
                m0_rgk = bcv(m0v.unsqueeze(2), (P, rc, 3, K0))
                pxys_rgk = bcv(PXYS.rearrange("p (g k) -> p g k", k=K0)
                               .unsqueeze(1), (P, rc, 3, K0))
                GP.tensor_tensor(out=tA3v, in0=m0_rgk, in1=pxys_rgk, op=OP.mult)
                yield
                VE.tensor_reduce(out=tA3rv, in_=tA3v, axis=AX.X, op=OP.add)
                yield
                cx = tA3rv[:, :, 0:1].squeeze(2)
                cy = tA3rv[:, :, 1:2].squeeze(2)
                c_f = tA3rv[:, :, 2:3].squeeze(2)

                d2b = rkpool.tile([P, RK], F32, tag="d2b")
                VE.scalar_tensor_tensor(out=d2b, in0=m0, scalar=BIG, in1=d2,
                                        op0=OP.mult, op1=OP.add)
                yield
                dmin2 = opool.tile([P, rc], F32, tag="dmin2")
                d2bv = d2b.rearrange("p (r k) -> p r k", k=K0)
                VE.tensor_reduce(out=dmin2, in_=d2bv, axis=AX.X, op=OP.min)
                yield
                dmin2_rk = bcv(dmin2.unsqueeze(2), (P, rc, K0))
                tA = rkpool.tile([P, RK], F32, tag="tA")
                tAv = tA.rearrange("p (r k) -> p r k", k=K0)
                i8_rk = bcv(IOTA8.unsqueeze(1), (P, rc, K0))
                VE.tensor_tensor(out=tAv, in0=d2bv, in1=dmin2_rk, op=OP.is_equal)
                yield
                GP.tensor_tensor(out=tAv, in0=tAv, in1=i8_rk, op=OP.mult)
                yield
                o1_f = opool.tile([P, rc], F32, tag="o1_f")
                VE.tensor_reduce(out=o1_f, in_=tAv, axis=AX.X, op=OP.add)
                yield

                v0x = rkpool.tile([P, RK], F32, tag="v0x")
                v0y = rkpool.tile([P, RK], F32, tag="v0y")
                d00 = rkpool.tile([P, RK], F32, tag="d00")
                d02 = rkpool.tile([P, RK], F32, tag="d02")
                tB = rkpool.tile([P, RK], F32, tag="tB")
                v0xv = v0x.rearrange("p (r k) -> p r k", k=K0)
                v0yv = v0y.rearrange("p (r k) -> p r k", k=K0)
                d02v = d02.rearrange("p (r k) -> p r k", k=K0)
                tBv = tB.rearrange("p (r k) -> p r k", k=K0)
                cx_rk = bcv(cx.unsqueeze(2), (P, rc, K0))
                cy_rk = bcv(cy.unsqueeze(2), (P, rc, K0))
                VE.tensor_tensor(out=v0xv, in0=px_rk, in1=cx_rk, op=OP.subtract)
                yield
                GP.tensor_tensor(out=v0yv, in0=py_rk, in1=cy_rk, op=OP.subtract)
                yield
                v2x = opool.tile([P, rc], F32, tag="v2x")
                v2y = opool.tile([P, rc], F32, tag="v2y")
                VE.tensor_tensor(out=v2x, in0=TX[:, r0:r1], in1=cx, op=OP.subtract)
                VE.tensor_tensor(out=v2y, in0=TY[:, r0:r1], in1=cy, op=OP.subtract)
                yield
                SC.activation(out=d00, in_=v0x, func=AF.Square)
                yield
                SC.activation(out=tB, in_=v0y, func=AF.Square)
                yield
                VE.tensor_tensor(out=d00, in0=d00, in1=tB, op=OP.add)
                yield
                v2x_rk = bcv(v2x.unsqueeze(2), (P, rc, K0))
                v2y_rk = bcv(v2y.unsqueeze(2), (P, rc, K0))
                VE.tensor_tensor(out=d02v, in0=v0xv, in1=v2x_rk, op=OP.mult)
                yield
                GP.tensor_tensor(out=tBv, in0=v0yv, in1=v2y_rk, op=OP.mult)
                yield
                VE.tensor_tensor(out=d02, in0=d02, in1=tB, op=OP.add)
                yield

                def XI(t2):
                    return bcv(t2.rearrange("p (r k) -> p r k", k=K0).unsqueeze(3),
                               (P, rc, K0, K0))

                def XJ(t2):
                    return bcv(t2.rearrange("p (r k) -> p r k", k=K0).unsqueeze(2),
                               (P, rc, K0, K0))

                dot01 = ppool.tile([P, PP], F32, tag="dot01")
                pA = ppool.tile([P, PP], F32, tag="pA")
                pB = ppool.tile([P, PP], F32, tag="pB")
                w2t = ppool.tile([P, PP], F32, tag="w2t")
                w0t = ppool.tile([P, PP], F32, tag="w0t")
                inv = ppool.tile([P, PP], F32, tag="inv")
                dot01v = dot01.rearrange("p (r i j) -> p r i j", i=K0, j=K0)
                pAv = pA.rearrange("p (r i j) -> p r i j", i=K0, j=K0)
                pBv = pB.rearrange("p (r i j) -> p r i j", i=K0, j=K0)

                VE.tensor_tensor(out=dot01v, in0=XI(v0x), in1=XJ(v0x), op=OP.mult)
                yield
                GP.tensor_tensor(out=pAv, in0=XI(v0y), in1=XJ(v0y), op=OP.mult)
                yield
                VE.tensor_tensor(out=dot01, in0=dot01, in1=pA, op=OP.add)
                yield
                GP.tensor_tensor(out=pAv, in0=XI(d00), in1=XJ(d00), op=OP.mult)
                yield
                SC.activation(out=pB, in_=dot01, func=AF.Square)
                yield
                VE.tensor_tensor(out=pA, in0=pA, in1=pB, op=OP.subtract)  # denom
                yield
                VE.reciprocal(out=inv, in_=pA)
                yield
                VE.tensor_scalar(out=inv, in0=inv, scalar1=BIG, scalar2=-BIG,
                                 op0=OP.min, op1=OP.max)
                yield
                VE.tensor_tensor(out=pAv, in0=XJ(d00), in1=XI(d02), op=OP.mult)
                yield
                GP.tensor_tensor(out=pBv, in0=dot01v, in1=XJ(d02), op=OP.mult)
                yield
                VE.tensor_tensor(out=w2t, in0=pA, in1=pB, op=OP.subtract)
                yield
                VE.tensor_tensor(out=w2t, in0=w2t, in1=inv, op=OP.mult)
                yield
                w2tv = w2t.rearrange("p (r i j) -> p r i j", i=K0, j=K0)
                w1t = w2t.rearrange("p (r i j) -> p r j i", i=K0, j=K0)
                VE.tensor_tensor(out=pAv, in0=w2tv, in1=w1t, op=OP.add)
                yield
                SC.activation(out=w0t, in_=pA, func=AF.Copy, bias=1.0, scale=-1.0)
                yield
                wm = ppool.tile([P, PP], F32, tag="wm")
                wmv = wm.rearrange("p (r i j) -> p r i j", i=K0, j=K0)
                VE.tensor_tensor(out=wmv, in0=w1t, in1=w2tv, op=OP.min)
                yield
                VE.tensor_tensor(out=wm, in0=wm, in1=w0t, op=OP.min)
                yield
                sq = ppool.tile([P, PP], F32, tag="sq")
                sr = ppool.tile([P, PP], F32, tag="sr")
                SC.activation(out=sr, in_=w0t, func=AF.Square)
                yield
                SC.activation(out=sq, in_=w2t, func=AF.Square)
                yield
                VE.tensor_tensor(out=sr, in0=sr, in1=sq, op=OP.max)
                yield
                srv = sr.rearrange("p (r i j) -> p r i j", i=K0, j=K0)
                VE.tensor_tensor(out=srv, in0=srv,
                                 in1=sq.rearrange("p (r i j) -> p r j i",
                                                  i=K0, j=K0), op=OP.max)
                yield
                VE.tensor_scalar(out=pB, in0=wm, scalar1=0.0, scalar2=BIG,
                                 op0=OP.is_le, op1=OP.mult)
                yield
                c_i = opool.tile([P, rc], I32, tag="c_i")
                VE.tensor_copy(out=c_i, in_=c_f)
                VE.tensor_scalar(out=c_i, in0=c_i, scalar1=-1, scalar2=31,
                                 op0=OP.mult, op1=OP.add)
                yield
                shf = ppool.tile([P, PP], F32, tag="inv")
                sh = shf.bitcast(I32)
                shv = sh.rearrange("p (r q) -> p r q", q=K2)
                VE.tensor_tensor(out=shv,
                                 in0=bcv(ipack.unsqueeze(1), (P, rc, K2)),
                                 in1=bcv(c_i.unsqueeze(2), (P, rc, K2)),
                                 op=OP.logical_shift_left)
                yield
                bf = ppool.tile([P, PP], F32, tag="pA")
                VE.tensor_scalar(out=bf, in0=sh, scalar1=0, scalar2=BIG,
                                 op0=OP.is_lt, op1=OP.mult)
                yield
                GP.tensor_tensor(out=bf, in0=bf, in1=pB, op=OP.add)
                yield
                score = ppool.tile([P, PP], F32, tag="score")
                VE.tensor_tensor(out=score, in0=sr, in1=bf, op=OP.max)
                yield

                scorev = score.rearrange("p (r q) -> p r q", q=K2)
                smin = opool.tile([P, rc], F32, tag="smin")
                VE.tensor_reduce(out=smin, in_=scorev, axis=AX.X, op=OP.min)
                yield
                smin_q = bcv(smin.unsqueeze(2), (P, rc, K2))
                oh = ppool.tile([P, PP], F32, tag="eqm")
                ohv_ = oh.rearrange("p (r q) -> p r q", q=K2)
                VE.tensor_tensor(out=ohv_, in0=scorev, in1=smin_q,
                                 op=OP.is_equal)
                yield
                # pidt = oh*(-1024) + (1024+iota): selected lanes = iota
                # exactly, others >= 1024; min-reduce = first argmin index
                i64b_q = bcv(IOTA64B.unsqueeze(1), (P, rc, K2))
                pidt = ppool.tile([P, PP], F32, tag="dot01")
                pidtv = pidt.rearrange("p (r q) -> p r q", q=K2)
                VE.scalar_tensor_tensor(out=pidtv, in0=ohv_, scalar=-1024.0,
                                        in1=i64b_q, op0=OP.mult, op1=OP.add)
                yield
                pidx = opool.tile([P, rc], F32, tag="pidx")
                VE.tensor_reduce(out=pidx, in_=pidtv, axis=AX.X, op=OP.min)
                yield
                oh4 = oh.rearrange("p (r i j) -> p r i j", i=K0, j=K0)
                Ga = ppool.tile([P, PP], F32, tag="sq")
                Gb = ppool.tile([P, PP], F32, tag="sr")
                GP.tensor_tensor(out=Ga.rearrange("p (r q) -> p r q", q=K2),
                                 in0=ohv_, in1=w2t.rearrange(
                    "p (r q) -> p r q", q=K2), op=OP.mult)
                yield
                GP.tensor_tensor(out=Gb.rearrange("p (r i j) -> p r i j",
                                                  i=K0, j=K0),
                                 in0=oh4, in1=w1t, op=OP.mult)
                yield
                wout = opool.tile([P, rc * 3], F32, tag="wout")
                woutv = wout.rearrange("p (r c) -> p r c", c=3)
                w2sel = woutv[:, :, 1]
                w1sel = woutv[:, :, 2]
                VE.tensor_reduce(out=w2sel, in_=Ga.rearrange(
                    "p (r q) -> p r q", q=K2), axis=AX.X, op=OP.add)
                yield
                VE.tensor_reduce(out=w1sel, in_=Gb.rearrange(
                    "p (r q) -> p r q", q=K2), axis=AX.X, op=OP.add)
                yield
                w0sel = woutv[:, :, 0]
                VE.tensor_tensor(out=w0sel, in0=w2sel, in1=w1sel, op=OP.add)
                yield
                SC.activation(out=w0sel, in_=w0sel, func=AF.Copy, bias=1.0,
                              scale=-1.0)
                yield
                pidxi = opool.tile([P, rc], I32, tag="pidxi")
                i_i = opool.tile([P, rc], I32, tag="i_i")
                j_i = opool.tile([P, rc], I32, tag="j_i")
                i_f = opool.tile([P, rc], F32, tag="i_f")
                j_f = opool.tile([P, rc], F32, tag="j_f")
                VE.tensor_copy(out=pidxi, in_=pidx)
                VE.tensor_scalar(out=i_i, in0=pidxi, scalar1=3, scalar2=None,
                                 op0=OP.arith_shift_right)
                VE.tensor_scalar(out=j_i, in0=pidxi, scalar1=7, scalar2=None,
                                 op0=OP.bitwise_and)
                yield
                VE.tensor_copy(out=i_f, in_=i_i)
                VE.tensor_copy(out=j_f, in_=j_i)
                yield
                fb = opool.tile([P, rc], F32, tag="fb")
                nfb = opool.tile([P, rc], F32, tag="nfb")
                VE.tensor_scalar(out=fb, in0=smin, scalar1=1.0e38, scalar2=None,
                                 op0=OP.is_ge)
                VE.tensor_scalar(out=nfb, in0=fb, scalar1=-1.0, scalar2=1.0,
                                 op0=OP.mult, op1=OP.add)
                yield
                VE.tensor_tensor(out=w0sel, in0=w0sel, in1=nfb, op=OP.mult)
                VE.tensor_tensor(out=w2sel, in0=w2sel, in1=nfb, op=OP.mult)
                VE.tensor_tensor(out=w1sel, in0=w1sel, in1=nfb, op=OP.mult)
                yield
                VE.copy_predicated(out=i_f, mask=fb.bitcast(I32), data=o1_f)
                VE.copy_predicated(out=j_f, mask=fb.bitcast(I32), data=o1_f)
                yield
                iout = opool.tile([P, rc * 3], F32, tag="iout")
                ioutv = iout.rearrange("p (r c) -> p r c", c=3)
                SC.copy(out=ioutv[:, :, 0], in_=c_f)
                SC.copy(out=ioutv[:, :, 1], in_=i_f)
                SC.copy(out=ioutv[:, :, 2], in_=j_f)
                yield
                nc.sync.dma_start(outw_d[v0_:v1_, r0:r1, :], woutv)
                nc.sync.dma_start(outi_d[v0_:v1_, r0:r1, :], ioutv)

            def run_rr(gens):
                alive = list(gens)
                while alive:
                    nxt = []
                    for g in alive:
                        try:
                            next(g)
                            nxt.append(g)
                        except StopIteration:
                            pass
                    alive = nxt

            # sliding-window round-robin: keep up to 3 independent
            # instruction streams in flight; a chunk stream is admitted only
            # once its v-tile's p0 stream has finished (p0s[vt] exists).
            work = []
            for vt in range(n_vt):
                work.append(("p0", vt + 1)) if False else None
            work = [("c", vt, ci) for vt in range(n_vt) for ci in range(n_rch)]
            p0q = [("p", vt) for vt in range(1, n_vt)]
            # merged queue: c(0,0), c(0,1), p(1), c(1,0), c(1,1), p(2), ...
            queue = []
            for vt in range(n_vt):
                queue.append(("c", vt, 0))
                queue.append(("c", vt, 1))
                if vt + 1 < n_vt:
                    queue.append(("p", vt + 1))

            def ready(item):
                return item[0] == "p" or item[1] in p0s

            def make(item):
                if item[0] == "p":
                    return p0_gen(item[1])
                return chunk_gen(item[1], item[2])

            run_rr([p0_gen(0)])
            active = []
            WINDOW = 3
            while queue or active:
                while len(active) < WINDOW and queue and ready(queue[0]):
                    active.append(make(queue.pop(0)))
                progressed = []
                for g in active:
                    try:
                        next(g)
                        progressed.append(g)
                    except StopIteration:
                        pass
                active = progressed
                if not active and queue:
                    # head not ready (p0 still queued behind?) — shouldn't
                    # happen with this queue order, but guard against stall
                    active.append(make(queue.pop(0)))

    nc.compile()
    return nc


def make_consts():
    iota8 = np.arange(K0, dtype=np.float32).reshape(1, K0)
    iota64 = np.arange(K2, dtype=np.float32).reshape(1, K2)
    iota64b = (1024.0 + np.arange(K2)).astype(np.float32).reshape(1, K2)
    pow2 = (2.0 ** np.arange(K0)).astype(np.float32).reshape(1, K0)
    return {"iota8": iota8, "iota64": iota64, "iota64b": iota64b,
            "pow2": pow2}


def make_in_maps(template, projections):
    template = np.ascontiguousarray(np.asarray(template, np.float32))
    projections = np.ascontiguousarray(np.asarray(projections, np.float32))
    consts = make_consts()
    tmplT = np.stack([template[..., 0].reshape(-1), template[..., 1].reshape(-1)])
    px_all = np.ascontiguousarray(projections[..., 0])
    py_all = np.ascontiguousarray(projections[..., 1])
    in_maps = []
    for c in range(N_CORES):
        pxc = px_all[c * VS:(c + 1) * VS]
        pyc = py_all[c * VS:(c + 1) * VS]
        pad = VSP - VS
        pxc = np.concatenate([pxc, np.broadcast_to(pxc[:1], (pad, K0))], 0)
        pyc = np.concatenate([pyc, np.broadcast_to(pyc[:1], (pad, K0))], 0)
        m = {"px": np.ascontiguousarray(pxc), "py": np.ascontiguousarray(pyc),
             "tmpl": tmplT}
        m.update(consts)
        in_maps.append(m)
    return in_maps


_NC_CACHE = {}


def kernel(template, projections, _want_time=False):
    from concourse.bass_utils import run_bass_kernel_spmd
    if "nc" not in _NC_CACHE:
        _NC_CACHE["nc"] = build_nc()
    nc = _NC_CACHE["nc"]
    in_maps = make_in_maps(template, projections)
    res = run_bass_kernel_spmd(nc, in_maps, core_ids=list(range(N_CORES)))
    ws, idxs = [], []
    for c in range(N_CORES):
        out = res.results[c]
        ws.append(out["outw"][:VS].reshape(VS, R, A, 3))
        idxs.append(out["outi"][:VS].reshape(VS, R, A, 3))
    w = np.concatenate(ws, 0).astype(np.float32)
    idx = np.rint(np.concatenate(idxs, 0)).astype(np.int32)
    if _want_time:
        return (w, idx), res
    return w, idx


# revision 16
# speedup vs baseline: 1.8804x; 1.0288x over previous
"""Trainium2 Bass kernel for BarycentricCoordinates (retrieval_knn).

v2's per-c Delaunay restructure + manual software pipelining: the two
r-chunks of a v-tile and the NEXT v-tile's per-c precompute are issued
round-robin, one instruction per slot, so consecutive ops on each engine
come from independent chains (hides DVE pipeline latency that serialized
v2 on HW). Arithmetic is identical to v2 (bitwise-matches the reference).
"""

import sys

sys.path.insert(0, "/opt/trn_rl_repo")

import numpy as np

import concourse.bass as bass
import concourse.bacc as bacc
import concourse.mybir as mybir
from concourse.tile import TileContext

F32 = mybir.dt.float32
I32 = mybir.dt.int32
OP = mybir.AluOpType
AF = mybir.ActivationFunctionType
AX = mybir.AxisListType

BIG = 2.0e38
N_CORES = 8
V_TOTAL = 5000
R, A, K0 = 5, 8, 8
RA = R * A
VS = V_TOTAL // N_CORES
P = 128
VSP = 640
RC = 20
K2 = 64
K3 = 512


def build_nc(vsp=VSP, rc=RC, ra=RA):
    nc = bacc.Bacc("TRN2", target_bir_lowering=False)
    n_vt = vsp // P
    n_rch = ra // rc

    px_d = nc.dram_tensor("px", (vsp, K0), F32, kind="ExternalInput")
    py_d = nc.dram_tensor("py", (vsp, K0), F32, kind="ExternalInput")
    tmpl_d = nc.dram_tensor("tmpl", (2, ra), F32, kind="ExternalInput")
    iota8_d = nc.dram_tensor("iota8", (1, K0), F32, kind="ExternalInput")
    iota64_d = nc.dram_tensor("iota64", (1, K2), F32, kind="ExternalInput")
    iota64b_d = nc.dram_tensor("iota64b", (1, K2), F32, kind="ExternalInput")
    pow2_d = nc.dram_tensor("pow2", (1, K0), F32, kind="ExternalInput")
    outw_d = nc.dram_tensor("outw", (vsp, ra, 3), F32, kind="ExternalOutput")
    outi_d = nc.dram_tensor("outi", (vsp, ra, 3), F32, kind="ExternalOutput")

    with TileContext(nc) as tc:
        VE = nc.vector
        GP = nc.gpsimd
        SC = nc.scalar
        PP = rc * K2
        RK = rc * K0

        with (
            tc.tile_pool(name="const", bufs=1) as cpool,
            tc.tile_pool(name="vt", bufs=2) as vpool,
            tc.tile_pool(name="det", bufs=1) as spool,
            tc.tile_pool(name="pair", bufs=2) as ppool,
            tc.tile_pool(name="rk", bufs=2) as rkpool,
            tc.tile_pool(name="small", bufs=2) as opool,
        ):
            TX = cpool.tile([P, ra], F32, tag="TX")
            TY = cpool.tile([P, ra], F32, tag="TY")
            IOTA8 = cpool.tile([P, K0], F32, tag="IOTA8")
            IOTA64 = cpool.tile([P, K2], F32, tag="IOTA64")
            IOTA64B = cpool.tile([P, K2], F32, tag="IOTA64B")
            POW2 = cpool.tile([P, K0], F32, tag="POW2")
            nc.sync.dma_start(TX, tmpl_d[0:1, :].to_broadcast((P, ra)))
            nc.sync.dma_start(TY, tmpl_d[1:2, :].to_broadcast((P, ra)))
            nc.sync.dma_start(IOTA8, iota8_d[0:1, :].to_broadcast((P, K0)))
            nc.sync.dma_start(IOTA64, iota64_d[0:1, :].to_broadcast((P, K2)))
            nc.sync.dma_start(IOTA64B, iota64b_d[0:1, :].to_broadcast((P, K2)))
            nc.sync.dma_start(POW2, pow2_d[0:1, :].to_broadcast((P, K0)))

            def bcv(ap, shape):
                return ap.to_broadcast(shape)

            p0s = {}

            def p0_gen(vt):
                """Per-v-tile precompute: s, b-tensors, U, det(c,ij,k), bit-pack."""
                v0_, v1_ = vt * P, (vt + 1) * P
                px = vpool.tile([P, K0], F32, tag="px")
                py = vpool.tile([P, K0], F32, tag="py")
                nc.sync.dma_start(px, px_d[v0_:v1_, :])
                nc.sync.dma_start(py, py_d[v0_:v1_, :])
                yield
                s_ = vpool.tile([P, K0], F32, tag="s")
                t8 = vpool.tile([P, K0], F32, tag="t8")
                SC.activation(out=s_, in_=px, func=AF.Square)
                SC.activation(out=t8, in_=py, func=AF.Square)
                yield
                VE.tensor_tensor(out=s_, in0=s_, in1=t8, op=OP.add)
                yield
                PXYS = vpool.tile([P, 24], F32, tag="PXYS")
                SC.copy(out=PXYS[:, 0:8], in_=px)
                SC.copy(out=PXYS[:, 8:16], in_=py)
                SC.copy(out=PXYS[:, 16:24], in_=IOTA8)
                yield
                bx = vpool.tile([P, K2], F32, tag="bx")
                by = vpool.tile([P, K2], F32, tag="by")
                bs = vpool.tile([P, K2], F32, tag="bs")
                bxv = bx.rearrange("p (i k) -> p i k", k=K0)
                byv = by.rearrange("p (i k) -> p i k", k=K0)
                bsv = bs.rearrange("p (i k) -> p i k", k=K0)
                VE.tensor_tensor(out=bxv, in0=bcv(px.unsqueeze(2), (P, K0, K0)),
                                 in1=bcv(px.unsqueeze(1), (P, K0, K0)),
                                 op=OP.subtract)
                yield
                GP.tensor_tensor(out=byv, in0=bcv(py.unsqueeze(2), (P, K0, K0)),
                                 in1=bcv(py.unsqueeze(1), (P, K0, K0)),
                                 op=OP.subtract)
                yield
                VE.tensor_tensor(out=bsv, in0=bcv(s_.unsqueeze(2), (P, K0, K0)),
                                 in1=bcv(s_.unsqueeze(1), (P, K0, K0)),
                                 op=OP.subtract)
                yield

                def Bi(t):
                    return bcv(t.rearrange("p (i k) -> p i k", k=K0).unsqueeze(2),
                               (P, K0, K0, K0))

                def Bj(t):
                    return bcv(t.rearrange("p (j k) -> p j k", k=K0).unsqueeze(1),
                               (P, K0, K0, K0))

                U1 = vpool.tile([P, K3], F32, tag="U1")
                U2 = vpool.tile([P, K3], F32, tag="U2")
                U3 = vpool.tile([P, K3], F32, tag="U3")
                uA = vpool.tile([P, K3], F32, tag="uA")
                U1v = U1.rearrange("p (i j k) -> p i j k", j=K0, k=K0)
                U2v = U2.rearrange("p (i j k) -> p i j k", j=K0, k=K0)
                U3v = U3.rearrange("p (i j k) -> p i j k", j=K0, k=K0)
                uAv = uA.rearrange("p (i j k) -> p i j k", j=K0, k=K0)
                VE.tensor_tensor(out=U1v, in0=Bi(by), in1=Bj(bs), op=OP.mult)
                yield
                GP.tensor_tensor(out=uAv, in0=Bi(bs), in1=Bj(by), op=OP.mult)
                yield
                VE.tensor_tensor(out=U1, in0=U1, in1=uA, op=OP.subtract)
                yield
                GP.tensor_tensor(out=U2v, in0=Bi(bx), in1=Bj(bs), op=OP.mult)
                yield
                GP.tensor_tensor(out=uAv, in0=Bi(bs), in1=Bj(bx), op=OP.mult)
                yield
                VE.tensor_tensor(out=U2, in0=U2, in1=uA, op=OP.subtract)
                yield
                VE.tensor_tensor(out=U3v, in0=Bi(bx), in1=Bj(by), op=OP.mult)
                yield
                GP.tensor_tensor(out=uAv, in0=Bi(by), in1=Bj(bx), op=OP.mult)
                yield
                VE.tensor_tensor(out=U3, in0=U3, in1=uA, op=OP.subtract)
                yield

                det = spool.tile([P, K0 * K3], F32, tag="det")
                dtm = spool.tile([P, K0 * K3], F32, tag="dtm")
                detv = det.rearrange("p (c q k) -> p c q k", q=K2, k=K0)
                dtmv = dtm.rearrange("p (c q k) -> p c q k", q=K2, k=K0)

                def Ck(t):
                    return bcv(t.rearrange("p (c k) -> p c k", k=K0).unsqueeze(2),
                               (P, K0, K2, K0))

                def Uq(t):
                    return bcv(t.rearrange("p (q k) -> p q k", k=K0).unsqueeze(1),
                               (P, K0, K2, K0))

                VE.tensor_tensor(out=detv, in0=Ck(bx), in1=Uq(U1), op=OP.mult)
                yield
                GP.tensor_tensor(out=dtmv, in0=Ck(by), in1=Uq(U2), op=OP.mult)
                yield
                VE.tensor_tensor(out=det, in0=det, in1=dtm, op=OP.subtract)
                yield
                GP.tensor_tensor(out=dtmv, in0=Ck(bs), in1=Uq(U3), op=OP.mult)
                yield
                VE.tensor_tensor(out=det, in0=det, in1=dtm, op=OP.add)
                yield
                VD = vpool.tile([P, K3], F32, tag="VD")
                VDv = VD.rearrange("p (c q) -> p c q", q=K2)
                VE.tensor_reduce(out=VDv, in_=detv, axis=AX.X, op=OP.max)
                yield
                mpb = vpool.tile([P, K3], F32, tag="uA")
                mpbv = mpb.rearrange("p (c q) -> p c q", q=K2)
                VE.scalar_tensor_tensor(out=mpbv, in0=VDv, scalar=0.0,
                                        in1=bcv(POW2.unsqueeze(2), (P, K0, K2)),
                                        op0=OP.is_gt, op1=OP.mult)
                yield
                Mp = vpool.tile([P, K2], F32, tag="Mp")
                VE.tensor_reduce(out=Mp,
                                 in_=mpb.rearrange("p (c q) -> p q c", q=K2),
                                 axis=AX.X, op=OP.add)
                yield
                ipack = vpool.tile([P, K2], I32, tag="ipack")
                VE.tensor_copy(out=ipack, in_=Mp)
                yield
                p0s[vt] = dict(px=px, py=py, PXYS=PXYS, ipack=ipack)

            def chunk_gen(vt, rchunk):
                """Per-(v-tile, r-chunk): closest, weights, score, selection."""
                v0_, v1_ = vt * P, (vt + 1) * P
                S = p0s[vt]
                px, py, PXYS, ipack = S["px"], S["py"], S["PXYS"], S["ipack"]
                r0 = rchunk * rc
                r1 = r0 + rc

                d2 = rkpool.tile([P, RK], F32, tag="d2")
                tdx = rkpool.tile([P, RK], F32, tag="tdx")
                tdy = rkpool.tile([P, RK], F32, tag="tdy")
                d2v = d2.rearrange("p (r k) -> p r k", k=K0)
                tdxv = tdx.rearrange("p (r k) -> p r k", k=K0)
                tdyv = tdy.rearrange("p (r k) -> p r k", k=K0)
                px_rk = bcv(px.unsqueeze(1), (P, rc, K0))
                py_rk = bcv(py.unsqueeze(1), (P, rc, K0))
                tx_rk = bcv(TX[:, r0:r1].unsqueeze(2), (P, rc, K0))
                ty_rk = bcv(TY[:, r0:r1].unsqueeze(2), (P, rc, K0))
                VE.tensor_tensor(out=tdxv, in0=px_rk, in1=tx_rk, op=OP.subtract)
                yield
                GP.tensor_tensor(out=tdyv, in0=py_rk, in1=ty_rk, op=OP.subtract)
                yield
                SC.activation(out=tdx, in_=tdx, func=AF.Square)
                yield
                SC.activation(out=tdy, in_=tdy, func=AF.Square)
                yield
                VE.tensor_tensor(out=d2, in0=tdx, in1=tdy, op=OP.add)
                yield
                dmin = opool.tile([P, rc], F32, tag="dmin")
                VE.tensor_reduce(out=dmin, in_=d2v, axis=AX.X, op=OP.min)
                yield
                dmin_rk = bcv(dmin.unsqueeze(2), (P, rc, K0))
                m0 = rkpool.tile([P, RK], F32, tag="m0")
                m0v = m0.rearrange("p (r k) -> p r k", k=K0)
                VE.tensor_tensor(out=m0v, in0=d2v, in1=dmin_rk, op=OP.is_equal)
                yield
                tA3 = rkpool.tile([P, RK * 3], F32, tag="tA3")
                tA3v = tA3.rearrange("p (r g k) -> p r g k", g=3, k=K0)
                tA3r = rkpool.tile([P, rc * 3], F32, tag="tA3r")
                tA3rv = tA3r.rearrange("p (r g) -> p r g", g=3)
                m0_rgk = bcv(m0v.unsqueeze(2), (P, rc, 3, K0))
                pxys_rgk = bcv(PXYS.rearrange("p (g k) -> p g k", k=K0)
                               .unsqueeze(1), (P, rc, 3, K0))
                GP.tensor_tensor(out=tA3v, in0=m0_rgk, in1=pxys_rgk, op=OP.mult)
                yield
                VE.tensor_reduce(out=tA3rv, in_=tA3v, axis=AX.X, op=OP.add)
                yield
                cx = tA3rv[:, :, 0:1].squeeze(2)
                cy = tA3rv[:, :, 1:2].squeeze(2)
                c_f = tA3rv[:, :, 2:3].squeeze(2)

                d2b = rkpool.tile([P, RK], F32, tag="d2b")
                VE.scalar_tensor_tensor(out=d2b, in0=m0, scalar=BIG, in1=d2,
                                        op0=OP.mult, op1=OP.add)
                yield
                dmin2 = opool.tile([P, rc], F32, tag="dmin2")
                d2bv = d2b.rearrange("p (r k) -> p r k", k=K0)
                VE.tensor_reduce(out=dmin2, in_=d2bv, axis=AX.X, op=OP.min)
                yield
                dmin2_rk = bcv(dmin2.unsqueeze(2), (P, rc, K0))
                tA = rkpool.tile([P, RK], F32, tag="tA")
                tAv = tA.rearrange("p (r k) -> p r k", k=K0)
                i8_rk = bcv(IOTA8.unsqueeze(1), (P, rc, K0))
                VE.tensor_tensor(out=tAv, in0=d2bv, in1=dmin2_rk, op=OP.is_equal)
                yield
                GP.tensor_tensor(out=tAv, in0=tAv, in1=i8_rk, op=OP.mult)
                yield
                o1_f = opool.tile([P, rc], F32, tag="o1_f")
                VE.tensor_reduce(out=o1_f, in_=tAv, axis=AX.X, op=OP.add)
                yield

                v0x = rkpool.tile([P, RK], F32, tag="v0x")
                v0y = rkpool.tile([P, RK], F32, tag="v0y")
                d00 = rkpool.tile([P, RK], F32, tag="d00")
                d02 = rkpool.tile([P, RK], F32, tag="d02")
                tB = rkpool.tile([P, RK], F32, tag="tB")
                v0xv = v0x.rearrange("p (r k) -> p r k", k=K0)
                v0yv = v0y.rearrange("p (r k) -> p r k", k=K0)
                d02v = d02.rearrange("p (r k) -> p r k", k=K0)
                tBv = tB.rearrange("p (r k) -> p r k", k=K0)
                cx_rk = bcv(cx.unsqueeze(2), (P, rc, K0))
                cy_rk = bcv(cy.unsqueeze(2), (P, rc, K0))
                VE.tensor_tensor(out=v0xv, in0=px_rk, in1=cx_rk, op=OP.subtract)
                yield
                GP.tensor_tensor(out=v0yv, in0=py_rk, in1=cy_rk, op=OP.subtract)
                yield
                v2x = opool.tile([P, rc], F32, tag="v2x")
                v2y = opool.tile([P, rc], F32, tag="v2y")
                VE.tensor_tensor(out=v2x, in0=TX[:, r0:r1], in1=cx, op=OP.subtract)
                VE.tensor_tensor(out=v2y, in0=TY[:, r0:r1], in1=cy, op=OP.subtract)
                yield
                SC.activation(out=d00, in_=v0x, func=AF.Square)
                yield
                SC.activation(out=tB, in_=v0y, func=AF.Square)
                yield
                VE.tensor_tensor(out=d00, in0=d00, in1=tB, op=OP.add)
                yield
                v2x_rk = bcv(v2x.unsqueeze(2), (P, rc, K0))
                v2y_rk = bcv(v2y.unsqueeze(2), (P, rc, K0))
                VE.tensor_tensor(out=d02v, in0=v0xv, in1=v2x_rk, op=OP.mult)
                yield
                GP.tensor_tensor(out=tBv, in0=v0yv, in1=v2y_rk, op=OP.mult)
                yield
                VE.tensor_tensor(out=d02, in0=d02, in1=tB, op=OP.add)
                yield

                def XI(t2):
                    return bcv(t2.rearrange("p (r k) -> p r k", k=K0).unsqueeze(3),
                               (P, rc, K0, K0))

                def XJ(t2):
                    return bcv(t2.rearrange("p (r k) -> p r k", k=K0).unsqueeze(2),
                               (P, rc, K0, K0))

                dot01 = ppool.tile([P, PP], F32, tag="dot01")
                pA = ppool.tile([P, PP], F32, tag="pA")
                pB = ppool.tile([P, PP], F32, tag="pB")
                w2t = ppool.tile([P, PP], F32, tag="w2t")
                w0t = ppool.tile([P, PP], F32, tag="w0t")
                inv = ppool.tile([P, PP], F32, tag="inv")
                dot01v = dot01.rearrange("p (r i j) -> p r i j", i=K0, j=K0)
                pAv = pA.rearrange("p (r i j) -> p r i j", i=K0, j=K0)
                pBv = pB.rearrange("p (r i j) -> p r i j", i=K0, j=K0)

                VE.tensor_tensor(out=dot01v, in0=XI(v0x), in1=XJ(v0x), op=OP.mult)
                yield
                GP.tensor_tensor(out=pAv, in0=XI(v0y), in1=XJ(v0y), op=OP.mult)
                yield
                VE.tensor_tensor(out=dot01, in0=dot01, in1=pA, op=OP.add)
                yield
                GP.tensor_tensor(out=pAv, in0=XI(d00), in1=XJ(d00), op=OP.mult)
                yield
                SC.activation(out=pB, in_=dot01, func=AF.Square)
                yield
                VE.tensor_tensor(out=pA, in0=pA, in1=pB, op=OP.subtract)  # denom
                yield
                VE.reciprocal(out=inv, in_=pA)
                yield
                VE.tensor_scalar(out=inv, in0=inv, scalar1=BIG, scalar2=-BIG,
                                 op0=OP.min, op1=OP.max)
                yield
                VE.tensor_tensor(out=pAv, in0=XJ(d00), in1=XI(d02), op=OP.mult)
                yield
                GP.tensor_tensor(out=pBv, in0=dot01v, in1=XJ(d02), op=OP.mult)
                yield
                VE.tensor_tensor(out=w2t, in0=pA, in1=pB, op=OP.subtract)
                yield
                VE.tensor_tensor(out=w2t, in0=w2t, in1=inv, op=OP.mult)
                yield
                w2tv = w2t.rearrange("p (r i j) -> p r i j", i=K0, j=K0)
                w1t = w2t.rearrange("p (r i j) -> p r j i", i=K0, j=K0)
                VE.tensor_tensor(out=pAv, in0=w2tv, in1=w1t, op=OP.add)
                yield
                SC.activation(out=w0t, in_=pA, func=AF.Copy, bias=1.0, scale=-1.0)
                yield
                wm = ppool.tile([P, PP], F32, tag="wm")
                wmv = wm.rearrange("p (r i j) -> p r i j", i=K0, j=K0)
                VE.tensor_tensor(out=wmv, in0=w1t, in1=w2tv, op=OP.min)
                yield
                VE.tensor_tensor(out=wm, in0=wm, in1=w0t, op=OP.min)
                yield
                sq = ppool.tile([P, PP], F32, tag="sq")
                sr = ppool.tile([P, PP], F32, tag="sr")
                SC.activation(out=sr, in_=w0t, func=AF.Square)
                yield
                SC.activation(out=sq, in_=w2t, func=AF.Square)
                yield
                VE.tensor_tensor(out=sr, in0=sr, in1=sq, op=OP.max)
                yield
                srv = sr.rearrange("p (r i j) -> p r i j", i=K0, j=K0)
                VE.tensor_tensor(out=srv, in0=srv,
                                 in1=sq.rearrange("p (r i j) -> p r j i",
                                                  i=K0, j=K0), op=OP.max)
                yield
                VE.tensor_scalar(out=pB, in0=wm, scalar1=0.0, scalar2=BIG,
                                 op0=OP.is_le, op1=OP.mult)
                yield
                c_i = opool.tile([P, rc], I32, tag="c_i")
                VE.tensor_copy(out=c_i, in_=c_f)
                VE.tensor_scalar(out=c_i, in0=c_i, scalar1=-1, scalar2=31,
                                 op0=OP.mult, op1=OP.add)
                yield
                shf = ppool.tile([P, PP], F32, tag="inv")
                sh = shf.bitcast(I32)
                shv = sh.rearrange("p (r q) -> p r q", q=K2)
                VE.tensor_tensor(out=shv,
                                 in0=bcv(ipack.unsqueeze(1), (P, rc, K2)),
                                 in1=bcv(c_i.unsqueeze(2), (P, rc, K2)),
                                 op=OP.logical_shift_left)
                yield
                bf = ppool.tile([P, PP], F32, tag="pA")
                VE.tensor_scalar(out=bf, in0=sh, scalar1=0, scalar2=BIG,
                                 op0=OP.is_lt, op1=OP.mult)
                yield
                GP.tensor_tensor(out=bf, in0=bf, in1=pB, op=OP.add)
                yield
                score = ppool.tile([P, PP], F32, tag="score")
                VE.tensor_tensor(out=score, in0=sr, in1=bf, op=OP.max)
                yield

                scorev = score.rearrange("p (r q) -> p r q", q=K2)
                smin = opool.tile([P, rc], F32, tag="smin")
                VE.tensor_reduce(out=smin, in_=scorev, axis=AX.X, op=OP.min)
                yield
                smin_q = bcv(smin.unsqueeze(2), (P, rc, K2))
                oh = ppool.tile([P, PP], F32, tag="eqm")
                ohv_ = oh.rearrange("p (r q) -> p r q", q=K2)
                VE.tensor_tensor(out=ohv_, in0=scorev, in1=smin_q,
                                 op=OP.is_equal)
                yield
                # pidt = oh*(-1024) + (1024+iota): selected lanes = iota
                # exactly, others >= 1024; min-reduce = first argmin index
                i64b_q = bcv(IOTA64B.unsqueeze(1), (P, rc, K2))
                pidt = ppool.tile([P, PP], F32, tag="dot01")
                pidtv = pidt.rearrange("p (r q) -> p r q", q=K2)
                VE.scalar_tensor_tensor(out=pidtv, in0=ohv_, scalar=-1024.0,
                                        in1=i64b_q, op0=OP.mult, op1=OP.add)
                yield
                pidx = opool.tile([P, rc], F32, tag="pidx")
                VE.tensor_reduce(out=pidx, in_=pidtv, axis=AX.X, op=OP.min)
                yield
                oh4 = oh.rearrange("p (r i j) -> p r i j", i=K0, j=K0)
                Ga = ppool.tile([P, PP], F32, tag="sq")
                Gb = ppool.tile([P, PP], F32, tag="sr")
                GP.tensor_tensor(out=Ga.rearrange("p (r q) -> p r q", q=K2),
                                 in0=ohv_, in1=w2t.rearrange(
                    "p (r q) -> p r q", q=K2), op=OP.mult)
                yield
                GP.tensor_tensor(out=Gb.rearrange("p (r i j) -> p r i j",
                                                  i=K0, j=K0),
                                 in0=oh4, in1=w1t, op=OP.mult)
                yield
                wout = opool.tile([P, rc * 3], F32, tag="wout")
                woutv = wout.rearrange("p (r c) -> p r c", c=3)
                w2sel = woutv[:, :, 1]
                w1sel = woutv[:, :, 2]
                VE.tensor_reduce(out=w2sel, in_=Ga.rearrange(
                    "p (r q) -> p r q", q=K2), axis=AX.X, op=OP.add)
                yield
                VE.tensor_reduce(out=w1sel, in_=Gb.rearrange(
                    "p (r q) -> p r q", q=K2), axis=AX.X, op=OP.add)
                yield
                w0sel = woutv[:, :, 0]
                VE.tensor_tensor(out=w0sel, in0=w2sel, in1=w1sel, op=OP.add)
                yield
                SC.activation(out=w0sel, in_=w0sel, func=AF.Copy, bias=1.0,
                              scale=-1.0)
                yield
                iout = opool.tile([P, rc * 3], F32, tag="iout")
                ioutv = iout.rearrange("p (r c) -> p r c", c=3)
                pidxi = opool.tile([P, rc], I32, tag="pidxi")
                i_i = opool.tile([P, rc], I32, tag="i_i")
                j_i = opool.tile([P, rc], I32, tag="j_i")
                VE.tensor_copy(out=pidxi, in_=pidx)
                VE.tensor_scalar(out=i_i, in0=pidxi, scalar1=3, scalar2=None,
                                 op0=OP.arith_shift_right)
                VE.tensor_scalar(out=j_i, in0=pidxi, scalar1=7, scalar2=None,
                                 op0=OP.bitwise_and)
                yield
                # decode straight into the interleaved output columns
                VE.tensor_copy(out=ioutv[:, :, 1], in_=i_i)
                VE.tensor_copy(out=ioutv[:, :, 2], in_=j_i)
                SC.copy(out=ioutv[:, :, 0], in_=c_f)
                yield
                fb = opool.tile([P, rc], F32, tag="fb")
                nfb = opool.tile([P, rc], F32, tag="nfb")
                VE.tensor_scalar(out=fb, in0=smin, scalar1=1.0e38, scalar2=None,
                                 op0=OP.is_ge)
                VE.tensor_scalar(out=nfb, in0=fb, scalar1=-1.0, scalar2=1.0,
                                 op0=OP.mult, op1=OP.add)
                yield
                # one fused fallback mask over all three weight columns
                VE.tensor_tensor(out=woutv, in0=woutv,
                                 in1=bcv(nfb.unsqueeze(2), (P, rc, 3)),
                                 op=OP.mult)
                yield
                # one predicated copy covers both neighbor-index columns
                VE.copy_predicated(out=ioutv[:, :, 1:3],
                                   mask=bcv(fb.unsqueeze(2),
                                            (P, rc, 2)).bitcast(I32),
                                   data=bcv(o1_f.unsqueeze(2), (P, rc, 2)))
                yield
                nc.sync.dma_start(outw_d[v0_:v1_, r0:r1, :], woutv)
                nc.sync.dma_start(outi_d[v0_:v1_, r0:r1, :], ioutv)

            def run_rr(gens):
                alive = list(gens)
                while alive:
                    nxt = []
                    for g in alive:
                        try:
                            next(g)
                            nxt.append(g)
                        except StopIteration:
                            pass
                    alive = nxt

            # sliding-window round-robin: keep up to 3 independent
            # instruction streams in flight; a chunk stream is admitted only
            # once its v-tile's p0 stream has finished (p0s[vt] exists).
            work = []
            for vt in range(n_vt):
                work.append(("p0", vt + 1)) if False else None
            work = [("c", vt, ci) for vt in range(n_vt) for ci in range(n_rch)]
            p0q = [("p", vt) for vt in range(1, n_vt)]
            # merged queue: c(0,0), c(0,1), p(1), c(1,0), c(1,1), p(2), ...
            queue = []
            for vt in range(n_vt):
                queue.append(("c", vt, 0))
                queue.append(("c", vt, 1))
                if vt + 1 < n_vt:
                    queue.append(("p", vt + 1))

            def ready(item):
                return item[0] == "p" or item[1] in p0s

            def make(item):
                if item[0] == "p":
                    return p0_gen(item[1])
                return chunk_gen(item[1], item[2])

            run_rr([p0_gen(0)])
            active = []
            WINDOW = 3
            while queue or active:
                while len(active) < WINDOW and queue and ready(queue[0]):
                    active.append(make(queue.pop(0)))
                progressed = []
                for g in active:
                    try:
                        next(g)
                        progressed.append(g)
                    except StopIteration:
                        pass
                active = progressed
                if not active and queue:
                    # head not ready (p0 still queued behind?) — shouldn't
                    # happen with this queue order, but guard against stall
                    active.append(make(queue.pop(0)))

    nc.compile()
    return nc


def make_consts():
    iota8 = np.arange(K0, dtype=np.float32).reshape(1, K0)
    iota64 = np.arange(K2, dtype=np.float32).reshape(1, K2)
    iota64b = (1024.0 + np.arange(K2)).astype(np.float32).reshape(1, K2)
    pow2 = (2.0 ** np.arange(K0)).astype(np.float32).reshape(1, K0)
    return {"iota8": iota8, "iota64": iota64, "iota64b": iota64b,
            "pow2": pow2}


def make_in_maps(template, projections):
    template = np.ascontiguousarray(np.asarray(template, np.float32))
    projections = np.ascontiguousarray(np.asarray(projections, np.float32))
    consts = make_consts()
    tmplT = np.stack([template[..., 0].reshape(-1), template[..., 1].reshape(-1)])
    px_all = np.ascontiguousarray(projections[..., 0])
    py_all = np.ascontiguousarray(projections[..., 1])
    in_maps = []
    for c in range(N_CORES):
        pxc = px_all[c * VS:(c + 1) * VS]
        pyc = py_all[c * VS:(c + 1) * VS]
        pad = VSP - VS
        pxc = np.concatenate([pxc, np.broadcast_to(pxc[:1], (pad, K0))], 0)
        pyc = np.concatenate([pyc, np.broadcast_to(pyc[:1], (pad, K0))], 0)
        m = {"px": np.ascontiguousarray(pxc), "py": np.ascontiguousarray(pyc),
             "tmpl": tmplT}
        m.update(consts)
        in_maps.append(m)
    return in_maps


_NC_CACHE = {}


def kernel(template, projections, _want_time=False):
    from concourse.bass_utils import run_bass_kernel_spmd
    if "nc" not in _NC_CACHE:
        _NC_CACHE["nc"] = build_nc()
    nc = _NC_CACHE["nc"]
    in_maps = make_in_maps(template, projections)
    res = run_bass_kernel_spmd(nc, in_maps, core_ids=list(range(N_CORES)))
    ws, idxs = [], []
    for c in range(N_CORES):
        out = res.results[c]
        ws.append(out["outw"][:VS].reshape(VS, R, A, 3))
        idxs.append(out["outi"][:VS].reshape(VS, R, A, 3))
    w = np.concatenate(ws, 0).astype(np.float32)
    idx = np.rint(np.concatenate(idxs, 0)).astype(np.int32)
    if _want_time:
        return (w, idx), res
    return w, idx


# revision 17
# speedup vs baseline: 2.3418x; 1.2454x over previous
"""Trainium2 Bass kernel for BarycentricCoordinates (retrieval_knn).

v2's per-c Delaunay restructure + manual software pipelining: the two
r-chunks of a v-tile and the NEXT v-tile's per-c precompute are issued
round-robin, one instruction per slot, so consecutive ops on each engine
come from independent chains (hides DVE pipeline latency that serialized
v2 on HW). Arithmetic is identical to v2 (bitwise-matches the reference).
"""

import sys

sys.path.insert(0, "/opt/trn_rl_repo")

import numpy as np

import concourse.bass as bass
import concourse.bacc as bacc
import concourse.mybir as mybir
from concourse.tile import TileContext

F32 = mybir.dt.float32
I32 = mybir.dt.int32
OP = mybir.AluOpType
AF = mybir.ActivationFunctionType
AX = mybir.AxisListType

BIG = 2.0e38
N_CORES = 8
V_TOTAL = 5000
R, A, K0 = 5, 8, 8
RA = R * A
VS = V_TOTAL // N_CORES
P = 128
VSP = 640
RC = 20
K2 = 64
K3 = 512


def build_nc(vsp=VSP, rc=RC, ra=RA):
    nc = bacc.Bacc("TRN2", target_bir_lowering=False)
    n_vt = vsp // P
    n_rch = ra // rc

    px_d = nc.dram_tensor("px", (vsp, K0), F32, kind="ExternalInput")
    py_d = nc.dram_tensor("py", (vsp, K0), F32, kind="ExternalInput")
    tmpl_d = nc.dram_tensor("tmpl", (2, ra), F32, kind="ExternalInput")
    iota8_d = nc.dram_tensor("iota8", (1, K0), F32, kind="ExternalInput")
    iota64_d = nc.dram_tensor("iota64", (1, K2), F32, kind="ExternalInput")
    iota64b_d = nc.dram_tensor("iota64b", (1, K2), F32, kind="ExternalInput")
    pow2_d = nc.dram_tensor("pow2", (1, K0), F32, kind="ExternalInput")
    outw_d = nc.dram_tensor("outw", (vsp, ra, 3), F32, kind="ExternalOutput")
    outi_d = nc.dram_tensor("outi", (vsp, ra, 3), F32, kind="ExternalOutput")

    with TileContext(nc) as tc:
        VE = nc.vector
        GP = nc.gpsimd
        SC = nc.scalar
        PP = rc * K2
        RK = rc * K0

        with (
            tc.tile_pool(name="const", bufs=1) as cpool,
            tc.tile_pool(name="vt", bufs=2) as vpool,
            tc.tile_pool(name="det", bufs=1) as spool,
            tc.tile_pool(name="pair", bufs=2) as ppool,
            tc.tile_pool(name="rk", bufs=2) as rkpool,
            tc.tile_pool(name="small", bufs=2) as opool,
        ):
            TX = cpool.tile([P, ra], F32, tag="TX")
            TY = cpool.tile([P, ra], F32, tag="TY")
            IOTA8 = cpool.tile([P, K0], F32, tag="IOTA8")
            IOTA64 = cpool.tile([P, K2], F32, tag="IOTA64")
            IOTA64B = cpool.tile([P, K2], F32, tag="IOTA64B")
            POW2 = cpool.tile([P, K0], F32, tag="POW2")
            nc.sync.dma_start(TX, tmpl_d[0:1, :].to_broadcast((P, ra)))
            nc.sync.dma_start(TY, tmpl_d[1:2, :].to_broadcast((P, ra)))
            nc.sync.dma_start(IOTA8, iota8_d[0:1, :].to_broadcast((P, K0)))
            nc.sync.dma_start(IOTA64, iota64_d[0:1, :].to_broadcast((P, K2)))
            nc.sync.dma_start(IOTA64B, iota64b_d[0:1, :].to_broadcast((P, K2)))
            nc.sync.dma_start(POW2, pow2_d[0:1, :].to_broadcast((P, K0)))

            def bcv(ap, shape):
                return ap.to_broadcast(shape)

            p0s = {}

            def p0_gen(vt):
                """Per-v-tile precompute: s, b-tensors, U, det(c,ij,k), bit-pack."""
                v0_, v1_ = vt * P, (vt + 1) * P
                px = vpool.tile([P, K0], F32, tag="px", bufs=3)
                py = vpool.tile([P, K0], F32, tag="py", bufs=3)
                nc.sync.dma_start(px, px_d[v0_:v1_, :])
                nc.sync.dma_start(py, py_d[v0_:v1_, :])
                yield
                s_ = vpool.tile([P, K0], F32, tag="s")
                t8 = vpool.tile([P, K0], F32, tag="t8")
                SC.activation(out=s_, in_=px, func=AF.Square)
                SC.activation(out=t8, in_=py, func=AF.Square)
                yield
                VE.tensor_tensor(out=s_, in0=s_, in1=t8, op=OP.add)
                yield
                PXYS = vpool.tile([P, 24], F32, tag="PXYS", bufs=3)
                SC.copy(out=PXYS[:, 0:8], in_=px)
                SC.copy(out=PXYS[:, 8:16], in_=py)
                SC.copy(out=PXYS[:, 16:24], in_=IOTA8)
                yield
                bx = vpool.tile([P, K2], F32, tag="bx")
                by = vpool.tile([P, K2], F32, tag="by")
                bs = vpool.tile([P, K2], F32, tag="bs")
                bxv = bx.rearrange("p (i k) -> p i k", k=K0)
                byv = by.rearrange("p (i k) -> p i k", k=K0)
                bsv = bs.rearrange("p (i k) -> p i k", k=K0)
                VE.tensor_tensor(out=bxv, in0=bcv(px.unsqueeze(2), (P, K0, K0)),
                                 in1=bcv(px.unsqueeze(1), (P, K0, K0)),
                                 op=OP.subtract)
                yield
                GP.tensor_tensor(out=byv, in0=bcv(py.unsqueeze(2), (P, K0, K0)),
                                 in1=bcv(py.unsqueeze(1), (P, K0, K0)),
                                 op=OP.subtract)
                yield
                VE.tensor_tensor(out=bsv, in0=bcv(s_.unsqueeze(2), (P, K0, K0)),
                                 in1=bcv(s_.unsqueeze(1), (P, K0, K0)),
                                 op=OP.subtract)
                yield

                def Bi(t):
                    return bcv(t.rearrange("p (i k) -> p i k", k=K0).unsqueeze(2),
                               (P, K0, K0, K0))

                def Bj(t):
                    return bcv(t.rearrange("p (j k) -> p j k", k=K0).unsqueeze(1),
                               (P, K0, K0, K0))

                U1 = vpool.tile([P, K3], F32, tag="U1")
                U2 = vpool.tile([P, K3], F32, tag="U2")
                U3 = vpool.tile([P, K3], F32, tag="U3")
                uA = vpool.tile([P, K3], F32, tag="uA")
                U1v = U1.rearrange("p (i j k) -> p i j k", j=K0, k=K0)
                U2v = U2.rearrange("p (i j k) -> p i j k", j=K0, k=K0)
                U3v = U3.rearrange("p (i j k) -> p i j k", j=K0, k=K0)
                uAv = uA.rearrange("p (i j k) -> p i j k", j=K0, k=K0)
                VE.tensor_tensor(out=U1v, in0=Bi(by), in1=Bj(bs), op=OP.mult)
                yield
                GP.tensor_tensor(out=uAv, in0=Bi(bs), in1=Bj(by), op=OP.mult)
                yield
                VE.tensor_tensor(out=U1, in0=U1, in1=uA, op=OP.subtract)
                yield
                GP.tensor_tensor(out=U2v, in0=Bi(bx), in1=Bj(bs), op=OP.mult)
                yield
                GP.tensor_tensor(out=uAv, in0=Bi(bs), in1=Bj(bx), op=OP.mult)
                yield
                VE.tensor_tensor(out=U2, in0=U2, in1=uA, op=OP.subtract)
                yield
                VE.tensor_tensor(out=U3v, in0=Bi(bx), in1=Bj(by), op=OP.mult)
                yield
                GP.tensor_tensor(out=uAv, in0=Bi(by), in1=Bj(bx), op=OP.mult)
                yield
                VE.tensor_tensor(out=U3, in0=U3, in1=uA, op=OP.subtract)
                yield

                det = spool.tile([P, K0 * K3], F32, tag="det")
                dtm = spool.tile([P, K0 * K3], F32, tag="dtm")
                detv = det.rearrange("p (c q k) -> p c q k", q=K2, k=K0)
                dtmv = dtm.rearrange("p (c q k) -> p c q k", q=K2, k=K0)

                def Ck(t):
                    return bcv(t.rearrange("p (c k) -> p c k", k=K0).unsqueeze(2),
                               (P, K0, K2, K0))

                def Uq(t):
                    return bcv(t.rearrange("p (q k) -> p q k", k=K0).unsqueeze(1),
                               (P, K0, K2, K0))

                VE.tensor_tensor(out=detv, in0=Ck(bx), in1=Uq(U1), op=OP.mult)
                yield
                GP.tensor_tensor(out=dtmv, in0=Ck(by), in1=Uq(U2), op=OP.mult)
                yield
                VE.tensor_tensor(out=det, in0=det, in1=dtm, op=OP.subtract)
                yield
                GP.tensor_tensor(out=dtmv, in0=Ck(bs), in1=Uq(U3), op=OP.mult)
                yield
                VE.tensor_tensor(out=det, in0=det, in1=dtm, op=OP.add)
                yield
                VD = vpool.tile([P, K3], F32, tag="VD")
                VDv = VD.rearrange("p (c q) -> p c q", q=K2)
                VE.tensor_reduce(out=VDv, in_=detv, axis=AX.X, op=OP.max)
                yield
                mpb = vpool.tile([P, K3], F32, tag="uA")
                mpbv = mpb.rearrange("p (c q) -> p c q", q=K2)
                VE.scalar_tensor_tensor(out=mpbv, in0=VDv, scalar=0.0,
                                        in1=bcv(POW2.unsqueeze(2), (P, K0, K2)),
                                        op0=OP.is_gt, op1=OP.mult)
                yield
                Mp = vpool.tile([P, K2], F32, tag="Mp", bufs=3)
                VE.tensor_reduce(out=Mp,
                                 in_=mpb.rearrange("p (c q) -> p q c", q=K2),
                                 axis=AX.X, op=OP.add)
                yield
                ipack = vpool.tile([P, K2], I32, tag="ipack", bufs=3)
                VE.tensor_copy(out=ipack, in_=Mp)
                yield
                p0s[vt] = dict(px=px, py=py, PXYS=PXYS, ipack=ipack)

            def chunk_gen(vt, rchunk):
                """Per-(v-tile, r-chunk): closest, weights, score, selection."""
                v0_, v1_ = vt * P, (vt + 1) * P
                S = p0s[vt]
                px, py, PXYS, ipack = S["px"], S["py"], S["PXYS"], S["ipack"]
                r0 = rchunk * rc
                r1 = r0 + rc

                d2 = rkpool.tile([P, RK], F32, tag="d2")
                tdx = rkpool.tile([P, RK], F32, tag="tdx")
                tdy = rkpool.tile([P, RK], F32, tag="tdy")
                d2v = d2.rearrange("p (r k) -> p r k", k=K0)
                tdxv = tdx.rearrange("p (r k) -> p r k", k=K0)
                tdyv = tdy.rearrange("p (r k) -> p r k", k=K0)
                px_rk = bcv(px.unsqueeze(1), (P, rc, K0))
                py_rk = bcv(py.unsqueeze(1), (P, rc, K0))
                tx_rk = bcv(TX[:, r0:r1].unsqueeze(2), (P, rc, K0))
                ty_rk = bcv(TY[:, r0:r1].unsqueeze(2), (P, rc, K0))
                VE.tensor_tensor(out=tdxv, in0=px_rk, in1=tx_rk, op=OP.subtract)
                yield
                GP.tensor_tensor(out=tdyv, in0=py_rk, in1=ty_rk, op=OP.subtract)
                yield
                SC.activation(out=tdx, in_=tdx, func=AF.Square)
                yield
                SC.activation(out=tdy, in_=tdy, func=AF.Square)
                yield
                VE.tensor_tensor(out=d2, in0=tdx, in1=tdy, op=OP.add)
                yield
                dmin = opool.tile([P, rc], F32, tag="dmin")
                VE.tensor_reduce(out=dmin, in_=d2v, axis=AX.X, op=OP.min)
                yield
                dmin_rk = bcv(dmin.unsqueeze(2), (P, rc, K0))
                m0 = rkpool.tile([P, RK], F32, tag="m0")
                m0v = m0.rearrange("p (r k) -> p r k", k=K0)
                VE.tensor_tensor(out=m0v, in0=d2v, in1=dmin_rk, op=OP.is_equal)
                yield
                tA3 = rkpool.tile([P, RK * 3], F32, tag="tA3")
                tA3v = tA3.rearrange("p (r g k) -> p r g k", g=3, k=K0)
                tA3r = rkpool.tile([P, rc * 3], F32, tag="tA3r", bufs=4)
                tA3rv = tA3r.rearrange("p (r g) -> p r g", g=3)
                m0_rgk = bcv(m0v.unsqueeze(2), (P, rc, 3, K0))
                pxys_rgk = bcv(PXYS.rearrange("p (g k) -> p g k", k=K0)
                               .unsqueeze(1), (P, rc, 3, K0))
                GP.tensor_tensor(out=tA3v, in0=m0_rgk, in1=pxys_rgk, op=OP.mult)
                yield
                VE.tensor_reduce(out=tA3rv, in_=tA3v, axis=AX.X, op=OP.add)
                yield
                cx = tA3rv[:, :, 0:1].squeeze(2)
                cy = tA3rv[:, :, 1:2].squeeze(2)
                c_f = tA3rv[:, :, 2:3].squeeze(2)

                d2b = rkpool.tile([P, RK], F32, tag="d2b")
                VE.scalar_tensor_tensor(out=d2b, in0=m0, scalar=BIG, in1=d2,
                                        op0=OP.mult, op1=OP.add)
                yield
                dmin2 = opool.tile([P, rc], F32, tag="dmin2")
                d2bv = d2b.rearrange("p (r k) -> p r k", k=K0)
                VE.tensor_reduce(out=dmin2, in_=d2bv, axis=AX.X, op=OP.min)
                yield
                dmin2_rk = bcv(dmin2.unsqueeze(2), (P, rc, K0))
                tA = rkpool.tile([P, RK], F32, tag="tA")
                tAv = tA.rearrange("p (r k) -> p r k", k=K0)
                i8_rk = bcv(IOTA8.unsqueeze(1), (P, rc, K0))
                VE.tensor_tensor(out=tAv, in0=d2bv, in1=dmin2_rk, op=OP.is_equal)
                yield
                GP.tensor_tensor(out=tAv, in0=tAv, in1=i8_rk, op=OP.mult)
                yield
                o1_f = opool.tile([P, rc], F32, tag="o1_f", bufs=4)
                VE.tensor_reduce(out=o1_f, in_=tAv, axis=AX.X, op=OP.add)
                yield

                v0x = rkpool.tile([P, RK], F32, tag="v0x", bufs=4)
                v0y = rkpool.tile([P, RK], F32, tag="v0y", bufs=4)
                d00 = rkpool.tile([P, RK], F32, tag="d00", bufs=4)
                d02 = rkpool.tile([P, RK], F32, tag="d02", bufs=4)
                tB = rkpool.tile([P, RK], F32, tag="tB")
                v0xv = v0x.rearrange("p (r k) -> p r k", k=K0)
                v0yv = v0y.rearrange("p (r k) -> p r k", k=K0)
                d02v = d02.rearrange("p (r k) -> p r k", k=K0)
                tBv = tB.rearrange("p (r k) -> p r k", k=K0)
                cx_rk = bcv(cx.unsqueeze(2), (P, rc, K0))
                cy_rk = bcv(cy.unsqueeze(2), (P, rc, K0))
                VE.tensor_tensor(out=v0xv, in0=px_rk, in1=cx_rk, op=OP.subtract)
                yield
                GP.tensor_tensor(out=v0yv, in0=py_rk, in1=cy_rk, op=OP.subtract)
                yield
                v2x = opool.tile([P, rc], F32, tag="v2x")
                v2y = opool.tile([P, rc], F32, tag="v2y")
                VE.tensor_tensor(out=v2x, in0=TX[:, r0:r1], in1=cx, op=OP.subtract)
                VE.tensor_tensor(out=v2y, in0=TY[:, r0:r1], in1=cy, op=OP.subtract)
                yield
                SC.activation(out=d00, in_=v0x, func=AF.Square)
                yield
                SC.activation(out=tB, in_=v0y, func=AF.Square)
                yield
                VE.tensor_tensor(out=d00, in0=d00, in1=tB, op=OP.add)
                yield
                v2x_rk = bcv(v2x.unsqueeze(2), (P, rc, K0))
                v2y_rk = bcv(v2y.unsqueeze(2), (P, rc, K0))
                VE.tensor_tensor(out=d02v, in0=v0xv, in1=v2x_rk, op=OP.mult)
                yield
                GP.tensor_tensor(out=tBv, in0=v0yv, in1=v2y_rk, op=OP.mult)
                yield
                VE.tensor_tensor(out=d02, in0=d02, in1=tB, op=OP.add)
                yield

                def XI(t2):
                    return bcv(t2.rearrange("p (r k) -> p r k", k=K0).unsqueeze(3),
                               (P, rc, K0, K0))

                def XJ(t2):
                    return bcv(t2.rearrange("p (r k) -> p r k", k=K0).unsqueeze(2),
                               (P, rc, K0, K0))

                dot01 = ppool.tile([P, PP], F32, tag="dot01")
                pA = ppool.tile([P, PP], F32, tag="pA")
                pB = ppool.tile([P, PP], F32, tag="pB")
                w2t = ppool.tile([P, PP], F32, tag="w2t", bufs=3)
                w0t = ppool.tile([P, PP], F32, tag="w0t")
                inv = ppool.tile([P, PP], F32, tag="inv")
                dot01v = dot01.rearrange("p (r i j) -> p r i j", i=K0, j=K0)
                pAv = pA.rearrange("p (r i j) -> p r i j", i=K0, j=K0)
                pBv = pB.rearrange("p (r i j) -> p r i j", i=K0, j=K0)

                VE.tensor_tensor(out=dot01v, in0=XI(v0x), in1=XJ(v0x), op=OP.mult)
                yield
                GP.tensor_tensor(out=pAv, in0=XI(v0y), in1=XJ(v0y), op=OP.mult)
                yield
                VE.tensor_tensor(out=dot01, in0=dot01, in1=pA, op=OP.add)
                yield
                GP.tensor_tensor(out=pAv, in0=XI(d00), in1=XJ(d00), op=OP.mult)
                yield
                SC.activation(out=pB, in_=dot01, func=AF.Square)
                yield
                VE.tensor_tensor(out=pA, in0=pA, in1=pB, op=OP.subtract)  # denom
                yield
                VE.reciprocal(out=inv, in_=pA)
                yield
                VE.tensor_scalar(out=inv, in0=inv, scalar1=BIG, scalar2=-BIG,
                                 op0=OP.min, op1=OP.max)
                yield
                VE.tensor_tensor(out=pAv, in0=XJ(d00), in1=XI(d02), op=OP.mult)
                yield
                GP.tensor_tensor(out=pBv, in0=dot01v, in1=XJ(d02), op=OP.mult)
                yield
                VE.tensor_tensor(out=w2t, in0=pA, in1=pB, op=OP.subtract)
                yield
                VE.tensor_tensor(out=w2t, in0=w2t, in1=inv, op=OP.mult)
                yield
                w2tv = w2t.rearrange("p (r i j) -> p r i j", i=K0, j=K0)
                w1t = w2t.rearrange("p (r i j) -> p r j i", i=K0, j=K0)
                VE.tensor_tensor(out=pAv, in0=w2tv, in1=w1t, op=OP.add)
                yield
                SC.activation(out=w0t, in_=pA, func=AF.Copy, bias=1.0, scale=-1.0)
                yield
                wm = ppool.tile([P, PP], F32, tag="wm")
                wmv = wm.rearrange("p (r i j) -> p r i j", i=K0, j=K0)
                VE.tensor_tensor(out=wmv, in0=w1t, in1=w2tv, op=OP.min)
                yield
                VE.tensor_tensor(out=wm, in0=wm, in1=w0t, op=OP.min)
                yield
                sq = ppool.tile([P, PP], F32, tag="sq")
                sr = ppool.tile([P, PP], F32, tag="sr")
                SC.activation(out=sr, in_=w0t, func=AF.Square)
                yield
                SC.activation(out=sq, in_=w2t, func=AF.Square)
                yield
                VE.tensor_tensor(out=sr, in0=sr, in1=sq, op=OP.max)
                yield
                srv = sr.rearrange("p (r i j) -> p r i j", i=K0, j=K0)
                VE.tensor_tensor(out=srv, in0=srv,
                                 in1=sq.rearrange("p (r i j) -> p r j i",
                                                  i=K0, j=K0), op=OP.max)
                yield
                VE.tensor_scalar(out=pB, in0=wm, scalar1=0.0, scalar2=BIG,
                                 op0=OP.is_le, op1=OP.mult)
                yield
                c_i = opool.tile([P, rc], I32, tag="c_i")
                VE.tensor_copy(out=c_i, in_=c_f)
                VE.tensor_scalar(out=c_i, in0=c_i, scalar1=-1, scalar2=31,
                                 op0=OP.mult, op1=OP.add)
                yield
                shf = ppool.tile([P, PP], F32, tag="inv")
                sh = shf.bitcast(I32)
                shv = sh.rearrange("p (r q) -> p r q", q=K2)
                VE.tensor_tensor(out=shv,
                                 in0=bcv(ipack.unsqueeze(1), (P, rc, K2)),
                                 in1=bcv(c_i.unsqueeze(2), (P, rc, K2)),
                                 op=OP.logical_shift_left)
                yield
                bf = ppool.tile([P, PP], F32, tag="pA")
                VE.tensor_scalar(out=bf, in0=sh, scalar1=0, scalar2=BIG,
                                 op0=OP.is_lt, op1=OP.mult)
                yield
                GP.tensor_tensor(out=bf, in0=bf, in1=pB, op=OP.add)
                yield
                score = ppool.tile([P, PP], F32, tag="score")
                VE.tensor_tensor(out=score, in0=sr, in1=bf, op=OP.max)
                yield

                scorev = score.rearrange("p (r q) -> p r q", q=K2)
                smin = opool.tile([P, rc], F32, tag="smin", bufs=4)
                VE.tensor_reduce(out=smin, in_=scorev, axis=AX.X, op=OP.min)
                yield
                smin_q = bcv(smin.unsqueeze(2), (P, rc, K2))
                oh = ppool.tile([P, PP], F32, tag="eqm")
                ohv_ = oh.rearrange("p (r q) -> p r q", q=K2)
                VE.tensor_tensor(out=ohv_, in0=scorev, in1=smin_q,
                                 op=OP.is_equal)
                yield
                # pidt = oh*(-1024) + (1024+iota): selected lanes = iota
                # exactly, others >= 1024; min-reduce = first argmin index
                i64b_q = bcv(IOTA64B.unsqueeze(1), (P, rc, K2))
                pidt = ppool.tile([P, PP], F32, tag="dot01")
                pidtv = pidt.rearrange("p (r q) -> p r q", q=K2)
                VE.scalar_tensor_tensor(out=pidtv, in0=ohv_, scalar=-1024.0,
                                        in1=i64b_q, op0=OP.mult, op1=OP.add)
                yield
                pidx = opool.tile([P, rc], F32, tag="pidx")
                VE.tensor_reduce(out=pidx, in_=pidtv, axis=AX.X, op=OP.min)
                yield
                oh4 = oh.rearrange("p (r i j) -> p r i j", i=K0, j=K0)
                Ga = ppool.tile([P, PP], F32, tag="sq")
                Gb = ppool.tile([P, PP], F32, tag="sr")
                GP.tensor_tensor(out=Ga.rearrange("p (r q) -> p r q", q=K2),
                                 in0=ohv_, in1=w2t.rearrange(
                    "p (r q) -> p r q", q=K2), op=OP.mult)
                yield
                GP.tensor_tensor(out=Gb.rearrange("p (r i j) -> p r i j",
                                                  i=K0, j=K0),
                                 in0=oh4, in1=w1t, op=OP.mult)
                yield
                wout = opool.tile([P, rc * 3], F32, tag="wout")
                woutv = wout.rearrange("p (r c) -> p r c", c=3)
                w2sel = woutv[:, :, 1]
                w1sel = woutv[:, :, 2]
                VE.tensor_reduce(out=w2sel, in_=Ga.rearrange(
                    "p (r q) -> p r q", q=K2), axis=AX.X, op=OP.add)
                yield
                VE.tensor_reduce(out=w1sel, in_=Gb.rearrange(
                    "p (r q) -> p r q", q=K2), axis=AX.X, op=OP.add)
                yield
                w0sel = woutv[:, :, 0]
                VE.tensor_tensor(out=w0sel, in0=w2sel, in1=w1sel, op=OP.add)
                yield
                SC.activation(out=w0sel, in_=w0sel, func=AF.Copy, bias=1.0,
                              scale=-1.0)
                yield
                iout = opool.tile([P, rc * 3], F32, tag="iout")
                ioutv = iout.rearrange("p (r c) -> p r c", c=3)
                pidxi = opool.tile([P, rc], I32, tag="pidxi")
                i_i = opool.tile([P, rc], I32, tag="i_i")
                j_i = opool.tile([P, rc], I32, tag="j_i")
                VE.tensor_copy(out=pidxi, in_=pidx)
                VE.tensor_scalar(out=i_i, in0=pidxi, scalar1=3, scalar2=None,
                                 op0=OP.arith_shift_right)
                VE.tensor_scalar(out=j_i, in0=pidxi, scalar1=7, scalar2=None,
                                 op0=OP.bitwise_and)
                yield
                # decode straight into the interleaved output columns
                VE.tensor_copy(out=ioutv[:, :, 1], in_=i_i)
                VE.tensor_copy(out=ioutv[:, :, 2], in_=j_i)
                SC.copy(out=ioutv[:, :, 0], in_=c_f)
                yield
                fb = opool.tile([P, rc], F32, tag="fb")
                nfb = opool.tile([P, rc], F32, tag="nfb")
                VE.tensor_scalar(out=fb, in0=smin, scalar1=1.0e38, scalar2=None,
                                 op0=OP.is_ge)
                VE.tensor_scalar(out=nfb, in0=fb, scalar1=-1.0, scalar2=1.0,
                                 op0=OP.mult, op1=OP.add)
                yield
                # one fused fallback mask over all three weight columns
                VE.tensor_tensor(out=woutv, in0=woutv,
                                 in1=bcv(nfb.unsqueeze(2), (P, rc, 3)),
                                 op=OP.mult)
                yield
                # one predicated copy covers both neighbor-index columns
                VE.copy_predicated(out=ioutv[:, :, 1:3],
                                   mask=bcv(fb.unsqueeze(2),
                                            (P, rc, 2)).bitcast(I32),
                                   data=bcv(o1_f.unsqueeze(2), (P, rc, 2)))
                yield
                nc.sync.dma_start(outw_d[v0_:v1_, r0:r1, :], woutv)
                nc.sync.dma_start(outi_d[v0_:v1_, r0:r1, :], ioutv)

            def run_rr(gens):
                alive = list(gens)
                while alive:
                    nxt = []
                    for g in alive:
                        try:
                            next(g)
                            nxt.append(g)
                        except StopIteration:
                            pass
                    alive = nxt

            # sliding-window round-robin: keep up to 3 independent
            # instruction streams in flight; a chunk stream is admitted only
            # once its v-tile's p0 stream has finished (p0s[vt] exists).
            work = []
            for vt in range(n_vt):
                work.append(("p0", vt + 1)) if False else None
            work = [("c", vt, ci) for vt in range(n_vt) for ci in range(n_rch)]
            p0q = [("p", vt) for vt in range(1, n_vt)]
            # merged queue: c(0,0), c(0,1), p(1), c(1,0), c(1,1), p(2), ...
            queue = []
            for vt in range(n_vt):
                queue.append(("c", vt, 0))
                queue.append(("c", vt, 1))
                if vt + 1 < n_vt:
                    queue.append(("p", vt + 1))

            def ready(item):
                return item[0] == "p" or item[1] in p0s

            def make(item):
                if item[0] == "p":
                    return p0_gen(item[1])
                return chunk_gen(item[1], item[2])

            run_rr([p0_gen(0)])
            active = []
            WINDOW = 3
            while queue or active:
                while len(active) < WINDOW and queue and ready(queue[0]):
                    active.append(make(queue.pop(0)))
                progressed = []
                for g in active:
                    try:
                        next(g)
                        progressed.append(g)
                    except StopIteration:
                        pass
                active = progressed
                if not active and queue:
                    # head not ready (p0 still queued behind?) — shouldn't
                    # happen with this queue order, but guard against stall
                    active.append(make(queue.pop(0)))

    nc.compile()
    return nc


def make_consts():
    iota8 = np.arange(K0, dtype=np.float32).reshape(1, K0)
    iota64 = np.arange(K2, dtype=np.float32).reshape(1, K2)
    iota64b = (1024.0 + np.arange(K2)).astype(np.float32).reshape(1, K2)
    pow2 = (2.0 ** np.arange(K0)).astype(np.float32).reshape(1, K0)
    return {"iota8": iota8, "iota64": iota64, "iota64b": iota64b,
            "pow2": pow2}


def make_in_maps(template, projections):
    template = np.ascontiguousarray(np.asarray(template, np.float32))
    projections = np.ascontiguousarray(np.asarray(projections, np.float32))
    consts = make_consts()
    tmplT = np.stack([template[..., 0].reshape(-1), template[..., 1].reshape(-1)])
    px_all = np.ascontiguousarray(projections[..., 0])
    py_all = np.ascontiguousarray(projections[..., 1])
    in_maps = []
    for c in range(N_CORES):
        pxc = px_all[c * VS:(c + 1) * VS]
        pyc = py_all[c * VS:(c + 1) * VS]
        pad = VSP - VS
        pxc = np.concatenate([pxc, np.broadcast_to(pxc[:1], (pad, K0))], 0)
        pyc = np.concatenate([pyc, np.broadcast_to(pyc[:1], (pad, K0))], 0)
        m = {"px": np.ascontiguousarray(pxc), "py": np.ascontiguousarray(pyc),
             "tmpl": tmplT}
        m.update(consts)
        in_maps.append(m)
    return in_maps


_NC_CACHE = {}


def kernel(template, projections, _want_time=False):
    from concourse.bass_utils import run_bass_kernel_spmd
    if "nc" not in _NC_CACHE:
        _NC_CACHE["nc"] = build_nc()
    nc = _NC_CACHE["nc"]
    in_maps = make_in_maps(template, projections)
    res = run_bass_kernel_spmd(nc, in_maps, core_ids=list(range(N_CORES)))
    ws, idxs = [], []
    for c in range(N_CORES):
        out = res.results[c]
        ws.append(out["outw"][:VS].reshape(VS, R, A, 3))
        idxs.append(out["outi"][:VS].reshape(VS, R, A, 3))
    w = np.concatenate(ws, 0).astype(np.float32)
    idx = np.rint(np.concatenate(idxs, 0)).astype(np.int32)
    if _want_time:
        return (w, idx), res
    return w, idx
